# revision 75
# baseline (speedup 1.0000x reference)
"""ArcticDecoderLayer on 8 TRN2 NeuronCores.

Sharding:
  - tokens: zigzag block-parallel (core i owns 128-token blocks {i, 15-i});
    attention, wo, residual MLP are token-parallel (weights replicated).
  - MoE: expert-parallel, 2 experts/core; routing/top-2 + token gather/scatter
    done host-side (part of shard/unshard), expert GEMMs on device.
  - One AllGather (K^T feature-major + V token-major, bf16) is the only
    collective; causal masking is data-driven (per-core exp-bias columns) so
    the SPMD graph is identical on all cores.

Restructure history (trace-driven, from the 800us v1 baseline):
  - Attention processes 4 GQA heads per matmul (shared K/V stationary,
    512-col streams) in 2 passes per kv-group (shallow qb0: 8 kbs+diag,
    deep qb1: 16 kbs+diag). Queries stored qb-major so causal masks stay
    per-partition exp biases; diag tri mask = exp * tri01 on vector.
  - Softmax denominators: every pass accumulates into its own partition row
    of ONE pinned psum bank (one-hot stationary), so a single [8,512]
    reciprocal + selector-matrix broadcasts normalize everything (a [1,N]
    reciprocal runs on a single DVE lane at ~7.6ns/elem - avoid).
  - MoE expert weights + expert inputs in fp8e4 (x32 pow-2 scale, dequant
    folded into the silu scale arg + ew), DoubleRow matmuls: [p,2,cols]
    k-pair APs on both operands. qkv/wo/w13/w2 stay bf16 (fp8 there costs
    1.4-1.7e-2 max-rel vs the 2e-2 gate; ws8+xg8 costs 1.15e-2).
  - KV AllGather split into shallow-half and deep-half collectives; qb0
    passes only need the shallow half. Packs/unpacks + small residents ride
    the gpsimd SWDGE ring so they never head-of-line-block the two HWDGE
    weight rings (sync=wsT/gu, scalar=xT/wqkv/xg).
  - Scalar engine stays exp-only inside attention (all silu before, table
    reloads cost 1.3us); attention PE filled with w2s thunks (vector-only
    consumes).
  - PSUM: "acct" (4 banks) gemm sweeps + score tiles + bcp; "aps" drain +
    "dps" den pin (2); "macct" (2) MoE thunks = 8 exactly.
"""
import numpy as np
import ml_dtypes

import concourse.bacc as bacc
import concourse.tile as tile
import concourse.mybir as mybir
from concourse.bass_utils import run_bass_kernel_spmd

F32 = mybir.dt.float32
BF16 = mybir.dt.bfloat16
FP8 = mybir.dt.float8e4
DR = mybir.MatmulPerfMode.DoubleRow
AF = mybir.ActivationFunctionType
WS8 = 32.0  # power-of-2 fp8 weight scale (dequant folded host-side)

H = 2048
NH = 16
NKV = 4
HD = 128
HALF = 64
I = 1024
E = 16
TOPK = 2
T = 2048
EPS = 1e-5
THETA = 10000.0
NC_ = 8
BLK = 128
NBLK = 16
TPC = 256  # tokens per core
EPC = 2  # experts per core
SCALE = HD ** -0.5
NEG = -30000.0
KQ = H // BLK  # 16
MI = I // BLK  # 8
GW = 4 * BLK  # 512 = one attention group-pass stream width (4 heads x 128)

TRACE = False
LAST_RESULT = None
_CACHE = {}

bf = lambda a: np.ascontiguousarray(np.asarray(a).astype(ml_dtypes.bfloat16))
f32 = lambda a: np.ascontiguousarray(a, dtype=np.float32)
f8 = lambda a: np.ascontiguousarray(np.asarray(a).astype(ml_dtypes.float8_e4m3fn))


def _slot(kb):
    c = min(kb, NBLK - 1 - kb)
    return 2 * c + (0 if kb < NC_ else 1)


def _build(cap):
    nc = bacc.Bacc("TRN2", target_bir_lowering=False, debug=False, num_devices=NC_)

    din = lambda name, shape, dt=BF16: nc.dram_tensor(name, shape, dt, kind="ExternalInput")
    xT_bf_d = din("xT_bf", [H, TPC])
    cos_d = din("cos_s", [HALF, TPC], F32)
    sin_d = din("sin_s", [HALF, TPC], F32)
    scol_d = din("s_col", [BLK, 2], F32)
    oh_d = din("ohtab", [BLK, 64])   # col p*8+j = (j==p): den -> psum row p
    sel_d = din("seltab", [8, 8 * BLK])  # [j, p*128+r] = (j==p): row-p bcast
    bias_d = din("bias", [2, NBLK, BLK], F32)
    tri4_d = din("tri4", [BLK, GW])  # bf16 0/1 causal mask tiled x4 heads
    ident_d = din("ident", [BLK, BLK])
    wqkv_d = din("wqkv", [H, NH * HD + 2 * NKV * HD])
    wo_d = din("wo", [NH * HD, H])
    w13_d = din("w13", [H, 2 * H])  # host-interleaved: chunk 2p=g_p, 2p+1=u_p
    w2_d = din("w2", [H, H])
    wsT_d = din("wsT", [EPC, H, 2 * I], FP8)  # host-interleaved g/u pairs
    w2sT_d = din("w2sT", [EPC, I, H])
    xg_d = din("xgT", [EPC, H, cap], FP8)
    ew_d = din("ew", [EPC, BLK, cap], F32)

    res_out_d = nc.dram_tensor("res_out", [H, TPC], F32, kind="ExternalOutput")
    moe_out_d = nc.dram_tensor("moe_out", [EPC, H, cap], BF16, kind="ExternalOutput")

    KG = 4  # contraction chunks per weight-stream DMA (tn=256 gemms)
    SW = 8  # m-chunks per sweep for TN=256 GEMMs (4 paired psum banks)

    with tile.TileContext(nc) as tc:
        with (
            tc.tile_pool(name="res", bufs=1) as res,
            tc.tile_pool(name="stream", bufs=2) as stream,
            tc.tile_pool(name="small", bufs=2) as small,
            tc.tile_pool(name="outp", bufs=2) as outp,
            tc.tile_pool(name="psum", bufs=4, space="PSUM") as psum,
            tc.tile_pool(name="dram", bufs=1, space="DRAM") as dram,
        ):
            eng_rr = [nc.sync, nc.scalar]

            # ---------------- early resident loads ----------------
            # xT first on sync (first qkv matmul needs chunk 0), cos/sin on
            # scalar; everything small goes on the gpsimd SWDGE ring so the
            # two HWDGE rings start streaming weights immediately.
            xT_sb = res.tile([BLK, KQ * TPC], BF16, tag="xT")
            nc.scalar.dma_start(xT_sb[:].rearrange("p (k t) -> p k t", k=KQ),
                               xT_bf_d.ap().rearrange("(k p) t -> p k t", p=BLK))
            cos2_sb = res.tile([HALF, 2 * TPC], F32, tag="cos")
            sin2_sb = res.tile([HALF, 2 * TPC], F32, tag="sin")
            scol_sb = res.tile([BLK, 2], F32, tag="scol")
            nc.gpsimd.dma_start(scol_sb[:], scol_d[:])
            oh_sb = res.tile([BLK, 64], BF16, tag="ohtab")
            nc.gpsimd.dma_start(oh_sb[:], oh_d[:])
            sel_sb = res.tile([8, 8 * BLK], BF16, tag="seltab")
            nc.gpsimd.dma_start(sel_sb[:], sel_d[:])
            bias_sb = res.tile([BLK, 2 * NBLK], F32, tag="bias")
            nc.gpsimd.dma_start(bias_sb[:], bias_d.ap().rearrange("a k p -> p (a k)"))
            tri4_sb = res.tile([BLK, GW], BF16, tag="tri4")
            nc.gpsimd.dma_start(tri4_sb[:], tri4_d[:])
            ident_sb = res.tile([BLK, BLK], BF16, tag="ident")
            nc.gpsimd.dma_start(ident_sb[:], ident_d[:])

            ones_row = res.tile([1, BLK], BF16, tag="onesr")
            nc.vector.memset(ones_row[:], 1.0)
            ones_cf = res.tile([BLK, 1], F32, tag="onescf")
            nc.vector.memset(ones_cf[:], 1.0)

            ew_sb = res.tile([BLK, EPC * cap], F32, tag="ew")
            xg_sb = res.tile([BLK, EPC * KQ * cap], FP8, tag="xg")
            attnU_sb = res.tile([BLK, NH * TPC], BF16, tag="attnU")

            # q stored qb-major: col = qb*2048 + h*128 + t
            q_sb = res.tile([BLK, NH * TPC], BF16, tag="q")
            k_sb = res.tile([BLK, NKV * TPC], BF16, tag="k")
            v_sb = res.tile([BLK, 2 * NKV * HD], BF16, tag="v")
            resid_sb = res.tile([BLK, KQ * TPC], F32, tag="resid")
            h2_sb = res.tile([BLK, KQ * TPC], BF16, tag="h2")
            gu_sb = res.tile([BLK, KQ * TPC], BF16, tag="gu")
            hm_sb = res.tile([BLK, EPC * MI * cap], BF16, tag="hm")

            kag_sb = res.tile([BLK, NBLK * NKV * BLK], BF16, tag="kag")
            vag_sb = res.tile([BLK, NBLK * NKV * BLK], BF16, tag="vag")

            # ============ generic streamed GEMM sweep ============
            # dr=True: fp8 DoubleRow — weights fp8, rhs_fn(kp) returns a
            # [128, 2, tn] k-pair AP; two 128-row k-tiles per matmul.
            def gemm(w_src, mcnt, kcnt, rhs_fn, tag, tn, consume, sweep, kg, tag_bufs,
                     sweep_starts=None, engs=None, dr=False):
                engs = engs or eng_rr
                pair = 2 * tn <= 512
                for s0 in (sweep_starts if sweep_starts is not None
                           else range(0, mcnt, sweep)):
                    ms = list(range(s0, min(s0 + sweep, mcnt)))
                    mw = len(ms)
                    if pair:
                        nt = (mw + 1) // 2
                        pts = [psum.tile([BLK, 2 * tn], F32, tag=tag, bufs=tag_bufs,
                                         name=f"pt{j}") for j in range(nt)]
                        paps = [pts[j // 2][:, (j % 2) * tn:(j % 2 + 1) * tn]
                                for j in range(mw)]
                    else:
                        pts = [psum.tile([BLK, tn], F32, tag=tag, bufs=tag_bufs,
                                         name=f"pt{j}") for j in range(mw)]
                        paps = [pts[j][:] for j in range(mw)]
                    for kg0 in range(0, kcnt, kg):
                        kgn = min(kg, kcnt - kg0)
                        wt = stream.tile([BLK, KG * SW * BLK], FP8 if dr else BF16,
                                         tag="wt8" if dr else "wt")
                        engs[(kg0 // kg) % len(engs)].dma_start(
                            wt[:, :kgn * mw * BLK].rearrange("p (k c) -> p k c", k=kgn),
                            w_src(kg0, kgn, ms[0] * BLK, (ms[-1] + 1) * BLK))
                        wtv = wt[:, :kgn * mw * BLK].rearrange("p (k c) -> p k c", k=kgn)
                        for kl in range(0, kgn, 2 if dr else 1):
                            k = kg0 + kl
                            for j in range(mw):
                                first = (j % 2 == 0) if pair else True
                                last = (j % 2 == 1 or j == mw - 1) if pair else True
                                if dr:
                                    nc.tensor.matmul(
                                        paps[j],
                                        wtv[:, kl:kl + 2, j * BLK:(j + 1) * BLK],
                                        rhs_fn(k // 2), start=(k == 0 and first),
                                        stop=(k == kcnt - 2 and last), perf_mode=DR)
                                else:
                                    nc.tensor.matmul(
                                        paps[j], wt[:, (kl * mw + j) * BLK:(kl * mw + j + 1) * BLK],
                                        rhs_fn(k), start=(k == 0 and first),
                                        stop=(k == kcnt - 1 and last))
                    consume(ms, paps, pts)

            # ---------------- QKV projection (feature-major out) ----------------
            def rope_pair(pt2, dst_ap):
                # pt2: [128, 512] psum pair (two head-chunks side by side);
                # dst_ap(half) -> AP matching [64, 512] column order of pt2.
                t1 = small.tile([HALF, 2 * TPC], F32, tag="r1")
                t2 = small.tile([HALF, 2 * TPC], F32, tag="r2")
                nc.vector.tensor_mul(t1[:], pt2[0:HALF, :], cos2_sb[:])
                nc.vector.tensor_mul(t2[:], pt2[HALF:BLK, :], sin2_sb[:])
                nc.vector.tensor_sub(dst_ap(0), t1[:], t2[:])
                t3 = small.tile([HALF, 2 * TPC], F32, tag="r1")
                t4 = small.tile([HALF, 2 * TPC], F32, tag="r2")
                nc.vector.tensor_mul(t3[:], pt2[HALF:BLK, :], cos2_sb[:])
                nc.vector.tensor_mul(t4[:], pt2[0:HALF, :], sin2_sb[:])
                nc.vector.tensor_add(dst_ap(1), t3[:], t4[:])

            def q_dst(m, half):
                # heads m, m+1; psum col order (h, qb0 128 | qb1 128);
                # q_sb col = qb*2048 + h*128 + t
                lo = half * HALF
                return q_sb[lo:lo + HALF, :] \
                    .rearrange("p (qb h t) -> p h qb t", qb=2, h=NH)[:, m:m + 2]

            def qkv_consume(ms, paps, pts):
                for jt, pt2 in enumerate(pts):
                    m = ms[2 * jt]
                    if m < NH:
                        rope_pair(pt2[:], lambda half, m=m: q_dst(m, half))
                    elif m < NH + NKV:
                        kvh = m - NH
                        rope_pair(pt2[:], lambda half, kvh=kvh: k_sb[
                            half * HALF:(half + 1) * HALF,
                            kvh * TPC:(kvh + 2) * TPC])
                    else:
                        for half_j in range(2):
                            kvh = m + half_j - NH - NKV
                            ps = pt2[:, half_j * TPC:(half_j + 1) * TPC]
                            vtmp = small.tile([BLK, TPC], BF16, tag="vtmp")
                            nc.vector.tensor_copy(vtmp[:], ps)
                            for tb in range(2):
                                ptt = psum.tile([BLK, BLK], BF16, tag="macct",
                                                bufs=2, name="ptt")
                                nc.tensor.transpose(ptt[:], vtmp[:, tb * BLK:(tb + 1) * BLK], ident_sb[:])
                                nc.vector.tensor_scalar_mul(
                                    v_sb[:, (tb * NKV + kvh) * BLK:(tb * NKV + kvh + 1) * BLK],
                                    ptt[:], scol_sb[:, tb:tb + 1])

            wqkv_src = lambda kg0, kgn, c0, c1: wqkv_d[kg0 * BLK:(kg0 + kgn) * BLK, c0:c1] \
                .rearrange("(k p) c -> p k c", p=BLK)
            # KV chunks first so the AllGather can launch early. qkv weights
            # stream on the scalar ring; wsT gets the sync ring so MoE gu
            # is never DMA-starved into the attention phase.
            qkv_rhs = lambda k: xT_sb[:, k * TPC:(k + 1) * TPC]
            for _rep in range(2):
                nc.scalar.dma_start(cos2_sb[:, _rep * TPC:(_rep + 1) * TPC], cos_d[:])
                nc.scalar.dma_start(sin2_sb[:, _rep * TPC:(_rep + 1) * TPC], sin_d[:])
            gemm(wqkv_src, NH + 2 * NKV, KQ, qkv_rhs, "acct", TPC, qkv_consume,
                 SW, KG, 4, sweep_starts=[16], engs=[nc.sync])

            # ---------------- KV AllGather (split shallow/deep) ----------------
            # Two collectives: each core's shallow block (kb 0..7 = even AG
            # slots) first, deep block second — the qb0 attention passes need
            # only the shallow half, so they can start while the deep half is
            # still on the wire.
            KSZ2 = NKV * BLK * BLK  # 65536 elems per half
            kv_sh = dram.tile([2, KSZ2], BF16)
            kv_dp = dram.tile([2, KSZ2], BF16)
            ag_sh = dram.tile([NC_, 2, KSZ2], BF16, addr_space="Shared")
            ag_dp = dram.tile([NC_, 2, KSZ2], BF16, addr_space="Shared")
            kv_loc = [kv_sh, kv_dp]
            for half in range(2):
                nc.gpsimd.dma_start(
                    kv_loc[half][0, :].rearrange("(h d t) -> d h t", h=NKV, d=BLK),
                    k_sb[:].rearrange("d (h t) -> d h t", h=NKV)[:, :, half * BLK:(half + 1) * BLK])
                nc.gpsimd.dma_start(
                    kv_loc[half][1, :].rearrange("(h t d) -> t h d", h=NKV, t=BLK),
                    v_sb[:].rearrange("t (b h d) -> b t h d", b=2, h=NKV)[half])
            nc.gpsimd.collective_compute(
                "AllGather", mybir.AluOpType.bypass,
                replica_groups=[list(range(NC_))],
                ins=[kv_sh[:]], outs=[ag_sh[:]])
            nc.gpsimd.collective_compute(
                "AllGather", mybir.AluOpType.bypass,
                replica_groups=[list(range(NC_))],
                ins=[kv_dp[:]], outs=[ag_dp[:]])
            # remaining qkv sweeps (q heads) overlap the collective
            gemm(wqkv_src, NH + 2 * NKV, KQ, qkv_rhs, "acct", TPC, qkv_consume,
                 SW, KG, 4, sweep_starts=[0, 8], engs=[nc.scalar])
            # deferred resident loads (needed by MoE / wo, not by qkv)
            for e in range(EPC):
                nc.scalar.dma_start(
                    xg_sb[:, e * KQ * cap:(e + 1) * KQ * cap].rearrange("p (k t) -> p k t", k=KQ),
                    xg_d[e].rearrange("(k p) t -> p k t", p=BLK))
                nc.scalar.dma_start(ew_sb[:, e * cap:(e + 1) * cap], ew_d[e])

            # ------------------- MoE sweeps (as thunks) -------------------
            def make_gu_thunks(e):
                thunks = []

                def gu_consume(ms, paps, pts, e=e):
                    for j in range(0, len(ms), 2):
                        p = (ms[0] + j) // 2
                        sg = small.tile([BLK, cap], BF16, tag="sg")
                        # psums are 32x (fp8 weight scale); silu(g) needs
                        # scale 1/32; the 32x on u is folded into ew.
                        nc.scalar.activation(sg[:], paps[j], AF.Silu, scale=1.0 / WS8)
                        nc.vector.tensor_mul(
                            hm_sb[:, (e * MI + p) * cap:(e * MI + p + 1) * cap],
                            sg[:], paps[j + 1])

                ws_src = lambda kg0, kgn, c0, c1, e=e: wsT_d[e, kg0 * BLK:(kg0 + kgn) * BLK, c0:c1] \
                    .rearrange("(k p) c -> p k c", p=BLK)
                xgv = xg_sb[:, e * KQ * cap:(e + 1) * KQ * cap] \
                    .rearrange("p (k t) -> p k t", k=KQ)
                rhs_e = lambda kp, xgv=xgv: xgv[:, 2 * kp:2 * kp + 2, :]
                for s0 in range(0, 2 * MI, 2):
                    thunks.append(lambda s0=s0, f=gu_consume, w=ws_src, r=rhs_e: gemm(
                        w, 2 * MI, KQ, r, "macct", cap, f, 2, 8, 2, sweep_starts=[s0],
                        engs=[nc.sync], dr=True))
                return thunks

            def make_w2s_thunks(e, tag, bufs, sweep):
                thunks = []

                def w2s_consume(ms, paps, pts, e=e):
                    for m, ps in zip(ms, paps):
                        mo = outp.tile([BLK, cap], BF16, tag="mo")
                        nc.vector.tensor_mul(mo[:], ps, ew_sb[:, e * cap:(e + 1) * cap])
                        eng_rr[m % 2].dma_start(moe_out_d[e, m * BLK:(m + 1) * BLK, :], mo[:])

                w2s_src = lambda kg0, kgn, c0, c1, e=e: w2sT_d[e, kg0 * BLK:(kg0 + kgn) * BLK, c0:c1] \
                    .rearrange("(k p) c -> p k c", p=BLK)
                for s0 in range(0, KQ, sweep):
                    thunks.append(lambda s0=s0, e=e, f=w2s_consume, w=w2s_src: gemm(
                        w, KQ, MI,
                        lambda k, e=e: hm_sb[:, (e * MI + k) * cap:(e * MI + k + 1) * cap],
                        tag, cap, f, sweep, 8, bufs, sweep_starts=[s0]))
                return thunks

            # both experts' gu inline before attention (silu done before any
            # exp act -> no act-table churn inside the attention phase)
            for th in make_gu_thunks(0):
                th()
            for th in make_gu_thunks(1):
                th()

            # unpack AG results on the gpsimd SWDGE ring (doesn't block the
            # HWDGE weight streams). slot = 2c+sub; attention maps kb->slot.
            ags = [ag_sh, ag_dp]
            for sub in range(2):
                for hh in range(NKV):
                    nc.gpsimd.dma_start(
                        kag_sb[:].rearrange("d (c s h t) -> s h d c t", c=NC_, s=2, h=NKV)[sub, hh],
                        ags[sub][:, 0, :].rearrange("c (h d t) -> h d c t", h=NKV, d=BLK)[hh])
                    nc.gpsimd.dma_start(
                        vag_sb[:].rearrange("t (c s h dd) -> s h t c dd", c=NC_, s=2, h=NKV)[sub, hh],
                        ags[sub][:, 1, :].rearrange("c (h t dd) -> h t c dd", h=NKV, t=BLK)[hh])

            # interleave thunks: w2s for both experts across attention
            # (consumes are vector-only -> scalar engine stays exp-only)
            fill = make_w2s_thunks(0, "macct", 2, 2) + make_w2s_thunks(1, "macct", 2, 2)

            # ---------------- attention: 4 kv-groups x 2 qb passes ----------------
            # pass (g, qb): 4 heads 4g..4g+3, queries = qb block (128 tokens).
            # dense kbs: qb0 -> kb 0..7, qb1 -> kb 0..15; diag = own block.
            # one pinned psum bank accumulates all 8 passes' denominators,
            # pass p landing on partition row p (one-hot stationary), so a
            # single [8,512] reciprocal serves the whole attention phase.
            dps = psum.tile([BLK, GW], F32, tag="dps", bufs=1)

            def attn_pass(g, qb, p):
                kbs = list(range(NC_ if qb == 0 else NBLK))
                nkb = len(kbs) + 1  # + diag
                qv = q_sb[:, qb * NH * BLK + 4 * g * BLK: qb * NH * BLK + 4 * (g + 1) * BLK]
                aps = psum.tile([BLK, GW], F32, tag="aps", bufs=1)
                for idx in range(nkb):
                    diag = idx == nkb - 1
                    sc = psum.tile([BLK, GW], F32, tag="acct", bufs=4, name="sc")
                    if diag:
                        kap = k_sb[:, g * TPC + qb * BLK: g * TPC + (qb + 1) * BLK]
                        vap = v_sb[:, (qb * NKV + g) * BLK:(qb * NKV + g + 1) * BLK]
                    else:
                        sl = _slot(kbs[idx])
                        kap = kag_sb[:, (sl * NKV + g) * BLK:(sl * NKV + g + 1) * BLK]
                        vap = vag_sb[:, (sl * NKV + g) * BLK:(sl * NKV + g + 1) * BLK]
                    nc.tensor.matmul(sc[:], kap, qv, start=True, stop=True)
                    pdn = small.tile([BLK, GW], BF16, tag="pdn", bufs=6)
                    if diag:
                        nc.scalar.activation(pdn[:], sc[:], AF.Exp, scale=SCALE)
                        nc.vector.tensor_mul(pdn[:], pdn[:], tri4_sb[:])
                    else:
                        nc.scalar.activation(pdn[:], sc[:], AF.Exp, scale=SCALE,
                                             bias=bias_sb[:, qb * NBLK + kbs[idx]:
                                                          qb * NBLK + kbs[idx] + 1])
                    nc.tensor.matmul(aps[:], vap, pdn[:], start=(idx == 0),
                                     stop=(idx == nkb - 1))
                    nc.tensor.matmul(dps[0:8, :], oh_sb[:, 8 * p:8 * p + 8], pdn[:],
                                     start=(p == 0 and idx == 0),
                                     stop=(p == 7 and idx == nkb - 1))
                # drain unnormalized; normalize batched after all passes
                nc.vector.tensor_copy(
                    attnU_sb[:, qb * NH * BLK + 4 * g * BLK: qb * NH * BLK + 4 * (g + 1) * BLK],
                    aps[:])

            # shallow (qb0) passes first — they only need ag_sh — then deep;
            # 12 w2s thunks inside attention, 4 left for the wo ramp.
            fi = 0
            for g in range(4):
                fill[fi](); fi += 1
                attn_pass(g, 0, g)
            for g in range(4):
                fill[fi](); fi += 1
                fill[fi](); fi += 1
                attn_pass(g, 1, 4 + g)

            # ---------------- batched softmax normalization ----------------
            rec_all = small.tile([8, GW], BF16, tag="rec")
            with nc.allow_low_precision(reason="softmax denom recip, ~2^-9 rel"):
                nc.vector.reciprocal(rec_all[:], dps[0:8, :])
            for p in range(8):
                g, qb = (p, 0) if p < 4 else (p - 4, 1)
                bcp = psum.tile([BLK, GW], F32, tag="acct", bufs=4, name="bcp")
                nc.tensor.matmul(bcp[:], sel_sb[:, p * BLK:(p + 1) * BLK],
                                 rec_all[:], start=True, stop=True)
                bcs = small.tile([BLK, GW], F32, tag="bcs")
                nc.vector.tensor_copy(bcs[:], bcp[:])
                au = attnU_sb[:, qb * NH * BLK + 4 * g * BLK:
                              qb * NH * BLK + 4 * (g + 1) * BLK]
                nc.vector.tensor_mul(au, au, bcs[:])

            # ---------------- wo + residual ----------------
            def wo_consume(ms, paps, pts):
                for jt, pt in enumerate(pts):
                    m0 = ms[2 * jt]
                    w = pt.shape[1]
                    nc.vector.tensor_add(resid_sb[:, m0 * TPC: m0 * TPC + w],
                                         pt[:], xT_sb[:, m0 * TPC: m0 * TPC + w])

            wo_src = lambda kg0, kgn, c0, c1: wo_d[kg0 * BLK:(kg0 + kgn) * BLK, c0:c1] \
                .rearrange("(k p) c -> p k c", p=BLK)
            wo_rhs = lambda k: attnU_sb[:].rearrange(
                "p (qb h t) -> p h qb t", qb=2, h=NH)[:, k]
            for i, s0 in enumerate(range(0, KQ, SW)):
                if fi < len(fill):
                    fill[fi](); fi += 1
                gemm(wo_src, KQ, KQ, wo_rhs, "acct", TPC, wo_consume, SW, KG, 4,
                     sweep_starts=[s0])
                if fi < len(fill):
                    fill[fi](); fi += 1
            while fi < len(fill):
                fill[fi](); fi += 1

            # ---------------- residual MLP norm scale ----------------
            ssq = psum.tile([BLK, TPC], F32, tag="macct", bufs=2, name="ssq")
            for k in range(KQ):
                sq = small.tile([BLK, TPC], F32, tag="sq")
                nc.vector.tensor_mul(sq[:], resid_sb[:, k * TPC:(k + 1) * TPC],
                                     resid_sb[:, k * TPC:(k + 1) * TPC])
                nc.tensor.matmul(ssq[0:1, :], ones_cf[:], sq[:],
                                 start=(k == 0), stop=(k == KQ - 1))
            vtmp2 = small.tile([1, TPC], F32, tag="vt")
            nc.vector.tensor_scalar(vtmp2[:], ssq[0:1, :], 1.0 / H, EPS,
                                    mybir.AluOpType.mult, mybir.AluOpType.add)
            st = small.tile([1, TPC], F32, tag="vt2")
            nc.scalar.activation(st[:], vtmp2[:], AF.Sqrt)
            s2r = small.tile([1, TPC], BF16, tag="vt3")
            with nc.allow_low_precision(reason="rms-norm scale recip, ~2^-9 rel"):
                nc.vector.reciprocal(s2r[:], st[:])
            s2p = psum.tile([BLK, TPC], F32, tag="macct", bufs=2, name="s2p")
            nc.tensor.matmul(s2p[:], ones_row[:], s2r[:], start=True, stop=True)
            s2s = small.tile([BLK, TPC], F32, tag="s2s")
            nc.scalar.activation(s2s[:], s2p[:], AF.Copy)
            for k in range(KQ):
                nc.vector.tensor_mul(h2_sb[:, k * TPC:(k + 1) * TPC],
                                     resid_sb[:, k * TPC:(k + 1) * TPC], s2s[:])

            # ---------------- w13 (interleaved g/u) + silu_and_mul ----------------
            def w13_consume(ms, paps, pts):
                for jt, pt in enumerate(pts):
                    p = ms[2 * jt] // 2
                    sg = small.tile([BLK, TPC], BF16, tag="sg13")
                    nc.scalar.activation(sg[:], pt[:, 0:TPC], AF.Silu)
                    nc.vector.tensor_mul(gu_sb[:, p * TPC:(p + 1) * TPC],
                                         sg[:], pt[:, TPC:2 * TPC])

            w13_src = lambda kg0, kgn, c0, c1: w13_d[kg0 * BLK:(kg0 + kgn) * BLK, c0:c1] \
                .rearrange("(k p) c -> p k c", p=BLK)
            gemm(w13_src, 2 * KQ, KQ, lambda k: h2_sb[:, k * TPC:(k + 1) * TPC],
                 "acct", TPC, w13_consume, SW, KG, 4)

            # ---------------- w2 + final out ----------------
            def w2_consume(ms, paps, pts):
                for jt, pt in enumerate(pts):
                    m0 = ms[2 * jt]
                    w = pt.shape[1]
                    fo = outp.tile([BLK, 2 * TPC], F32, tag="fo")
                    nc.vector.tensor_add(fo[:, :w], pt[:], resid_sb[:, m0 * TPC:m0 * TPC + w])
                    nc.sync.dma_start(
                        res_out_d.ap().rearrange("(m p) t -> p m t", p=BLK)[:, m0:m0 + w // TPC],
                        fo[:, :w].rearrange("p (m t) -> p m t", t=TPC))

            w2_src = lambda kg0, kgn, c0, c1: w2_d[kg0 * BLK:(kg0 + kgn) * BLK, c0:c1] \
                .rearrange("(k p) c -> p k c", p=BLK)
            gemm(w2_src, KQ, KQ, lambda k: gu_sb[:, k * TPC:(k + 1) * TPC],
                 "acct", TPC, w2_consume, SW, KG, 4)

    nc.compile()
    return nc


def _interleave_cols(w, half):
    # [rows, 2*half] -> column chunks reordered so chunk 2p=g_p, 2p+1=u_p
    rows = w.shape[0]
    g = w[:, :half].reshape(rows, half // BLK, BLK)
    u = w[:, half:].reshape(rows, half // BLK, BLK)
    out = np.empty((rows, 2 * (half // BLK), BLK), w.dtype)
    out[:, 0::2] = g
    out[:, 1::2] = u
    return out.reshape(rows, 2 * half // BLK * BLK)


def kernel(**inputs):
    global LAST_RESULT
    hidden = f32(inputs["hidden_states"])
    positions = np.asarray(inputs["positions"]).astype(np.float32)
    ln_in_w = f32(inputs["ln_in_w"])
    ln_post_w = f32(inputs["ln_post_w"])
    ln_res_w = f32(inputs["ln_res_w"])
    wqkv = f32(inputs["wqkv"])
    wo = f32(inputs["wo"])
    res_w13 = f32(inputs["res_w13"])
    res_w2 = f32(inputs["res_w2"])
    gate_w = f32(inputs["gate_w"])
    ws = f32(inputs["ws"])
    w2s = f32(inputs["w2s"])

    # ---- host prep (sharding) ----
    s = 1.0 / np.sqrt(np.mean(hidden * hidden, axis=1) + EPS)  # [T]
    x_norm = hidden * s[:, None]

    logits = (x_norm * ln_post_w) @ gate_w
    pr = np.exp(logits - logits.max(-1, keepdims=True))
    pr /= pr.sum(-1, keepdims=True)
    topi = np.argsort(-pr, axis=-1, kind="stable")[:, :TOPK]
    topw = np.take_along_axis(pr, topi, axis=-1)
    topw /= topw.sum(-1, keepdims=True)
    tok_lists = [np.where((topi == e).any(-1))[0] for e in range(E)]
    wts = [np.sum(np.where(topi[tl] == e, topw[tl], 0.0), -1).astype(np.float32)
           for e, tl in zip(range(E), tok_lists)]
    cap = max(128, -(-max(len(t) for t in tok_lists) // 32) * 32)
    assert cap <= 512, cap

    ck = cap
    if ck not in _CACHE:
        _CACHE[ck] = _build(cap)
    nc = _CACHE[ck]

    inv_freq = 1.0 / (THETA ** (np.arange(0, HD, 2, dtype=np.float32) / HD))
    ang = positions[:, None] * inv_freq
    cos_t, sin_t = np.cos(ang), np.sin(ang)

    tri01 = (np.arange(BLK)[None, :] >= np.arange(BLK)[:, None]).astype(np.float32)
    tri4 = np.tile(tri01, (1, 4))
    ident = np.eye(BLK, dtype=np.float32)
    ohtab = np.zeros((BLK, 64), np.float32)
    for p in range(8):
        ohtab[:, p * 8 + p] = 1.0
    seltab = np.zeros((8, 8 * BLK), np.float32)
    for p in range(8):
        seltab[p, p * BLK:(p + 1) * BLK] = 1.0

    wqkv_f = wqkv * ln_in_w[:, None]
    w13_f = _interleave_cols(res_w13 * ln_res_w[:, None], H)
    x_norm_post = x_norm * ln_post_w
    wsT = ws.transpose(0, 2, 1)  # [E, H, 2I]
    wsT_il = np.stack([_interleave_cols(wsT[e], I) for e in range(E)])
    w2sT = w2s.transpose(0, 2, 1)

    shared = {
        "tri4": bf(tri4), "ident": bf(ident),
        "ohtab": bf(ohtab), "seltab": bf(seltab),
        "wqkv": bf(wqkv_f), "wo": bf(wo), "w13": bf(w13_f), "w2": bf(res_w2),
    }

    in_maps = []
    own = [[i, NBLK - 1 - i] for i in range(NC_)]
    for i in range(NC_):
        toks = np.concatenate([np.arange(b * BLK, (b + 1) * BLK) for b in own[i]])
        xT = hidden[toks].T
        cs = (cos_t[toks] * s[toks, None]).T
        sn = (sin_t[toks] * s[toks, None]).T
        scol = np.stack([s[toks[:BLK]], s[toks[BLK:]]], axis=1)
        bias = np.zeros((2, NBLK, BLK), np.float32)
        b0, b1 = own[i]
        bias[0, b0:, :] = NEG
        bias[1, b1:, :] = NEG
        exps = [2 * i, 2 * i + 1]
        xg = np.zeros((EPC, H, cap), np.float32)
        ew = np.zeros((EPC, BLK, cap), np.float32)
        for j, e in enumerate(exps):
            n = len(tok_lists[e])
            xg[j, :, :n] = x_norm_post[tok_lists[e]].T
            # ws runs in fp8 with weights x32; u-path 32x folded here
            ew[j, :, :n] = wts[e][None, :] / WS8
        in_maps.append({
            "xT_bf": bf(xT),
            "cos_s": f32(cs), "sin_s": f32(sn), "s_col": f32(scol),
            "bias": bias,
            "wsT": f8(wsT_il[exps] * WS8),
            "w2sT": bf(w2sT[exps]),
            "xgT": f8(xg), "ew": ew,
            **shared,
        })

    res = run_bass_kernel_spmd(nc, in_maps, core_ids=list(range(NC_)), trace=TRACE)
    LAST_RESULT = res

    out = np.zeros((T, H), np.float32)
    for i in range(NC_):
        toks = np.concatenate([np.arange(b * BLK, (b + 1) * BLK) for b in own[i]])
        out[toks] = res.results[i]["res_out"].T
    for i in range(NC_):
        for j, e in enumerate((2 * i, 2 * i + 1)):
            tl = tok_lists[e]
            out[tl] += res.results[i]["moe_out"][j].astype(np.float32).T[:len(tl)]
    return out


# revision 77
# speedup vs baseline: 1.0257x; 1.0257x over previous
"""ArcticDecoderLayer on 8 TRN2 NeuronCores.

Sharding:
  - tokens: zigzag block-parallel (core i owns 128-token blocks {i, 15-i});
    attention, wo, residual MLP are token-parallel (weights replicated).
  - MoE: expert-parallel, 2 experts/core; routing/top-2 + token gather/scatter
    done host-side (part of shard/unshard), expert GEMMs on device.
  - One AllGather (K^T feature-major + V token-major, bf16) is the only
    collective; causal masking is data-driven (per-core exp-bias columns) so
    the SPMD graph is identical on all cores.

Restructure history (trace-driven, from the 800us v1 baseline):
  - Attention processes 4 GQA heads per matmul (shared K/V stationary,
    512-col streams) in 2 passes per kv-group (shallow qb0: 8 kbs+diag,
    deep qb1: 16 kbs+diag). Queries stored qb-major so causal masks stay
    per-partition exp biases; diag tri mask = exp * tri01 on vector.
  - Softmax denominators: every pass accumulates into its own partition row
    of ONE pinned psum bank (one-hot stationary), so a single [8,512]
    reciprocal + selector-matrix broadcasts normalize everything (a [1,N]
    reciprocal runs on a single DVE lane at ~7.6ns/elem - avoid).
  - MoE expert weights + expert inputs in fp8e4 (x32 pow-2 scale, dequant
    folded into the silu scale arg + ew), DoubleRow matmuls: [p,2,cols]
    k-pair APs on both operands. qkv/wo/w13/w2 stay bf16 (fp8 there costs
    1.4-1.7e-2 max-rel vs the 2e-2 gate; ws8+xg8 costs 1.15e-2).
  - KV AllGather split into shallow-half and deep-half collectives; qb0
    passes only need the shallow half. Packs/unpacks + small residents ride
    the gpsimd SWDGE ring so they never head-of-line-block the two HWDGE
    weight rings (sync=wsT/gu, scalar=xT/wqkv/xg).
  - Scalar engine stays exp-only inside attention (all silu before, table
    reloads cost 1.3us); attention PE filled with w2s thunks (vector-only
    consumes).
  - PSUM: "acct" (4 banks) gemm sweeps + score tiles + bcp; "aps" drain +
    "dps" den pin (2); "macct" (2) MoE thunks = 8 exactly.
"""
import numpy as np
import ml_dtypes

import concourse.bacc as bacc
import concourse.tile as tile
import concourse.mybir as mybir
from concourse.bass_utils import run_bass_kernel_spmd

F32 = mybir.dt.float32
BF16 = mybir.dt.bfloat16
FP8 = mybir.dt.float8e4
DR = mybir.MatmulPerfMode.DoubleRow
AF = mybir.ActivationFunctionType
WS8 = 32.0  # power-of-2 fp8 weight scale (dequant folded host-side)

H = 2048
NH = 16
NKV = 4
HD = 128
HALF = 64
I = 1024
E = 16
TOPK = 2
T = 2048
EPS = 1e-5
THETA = 10000.0
NC_ = 8
BLK = 128
NBLK = 16
TPC = 256  # tokens per core
EPC = 2  # experts per core
SCALE = HD ** -0.5
NEG = -30000.0
KQ = H // BLK  # 16
MI = I // BLK  # 8
GW = 4 * BLK  # 512 = one attention group-pass stream width (4 heads x 128)

TRACE = False
LAST_RESULT = None
_CACHE = {}

bf = lambda a: np.ascontiguousarray(np.asarray(a).astype(ml_dtypes.bfloat16))
f32 = lambda a: np.ascontiguousarray(a, dtype=np.float32)
f8 = lambda a: np.ascontiguousarray(np.asarray(a).astype(ml_dtypes.float8_e4m3fn))


def _slot(kb):
    c = min(kb, NBLK - 1 - kb)
    return 2 * c + (0 if kb < NC_ else 1)


def _build(cap):
    nc = bacc.Bacc("TRN2", target_bir_lowering=False, debug=False, num_devices=NC_)

    din = lambda name, shape, dt=BF16: nc.dram_tensor(name, shape, dt, kind="ExternalInput")
    xT_bf_d = din("xT_bf", [H, TPC])
    cos_d = din("cos_s", [HALF, TPC], F32)
    sin_d = din("sin_s", [HALF, TPC], F32)
    scol_d = din("s_col", [BLK, 2], F32)
    oh_d = din("ohtab", [BLK, 64])   # col p*8+j = (j==p): den -> psum row p
    sel_d = din("seltab", [8, 8 * BLK])  # [j, p*128+r] = (j==p): row-p bcast
    bias_d = din("bias", [2, NBLK, BLK], F32)
    tri4_d = din("tri4", [BLK, GW])  # bf16 0/1 causal mask tiled x4 heads
    ident_d = din("ident", [BLK, BLK])
    wqkv_d = din("wqkv", [H, NH * HD + 2 * NKV * HD])
    wo_d = din("wo", [NH * HD, H])
    w13_d = din("w13", [H, 2 * H])  # host-interleaved: chunk 2p=g_p, 2p+1=u_p
    w2_d = din("w2", [H, H])
    wsT_d = din("wsT", [EPC, H, 2 * I], FP8)  # host-interleaved g/u pairs
    w2sT_d = din("w2sT", [EPC, I, H])
    xg_d = din("xgT", [EPC, H, cap], FP8)
    ew_d = din("ew", [EPC, BLK, cap], F32)

    res_out_d = nc.dram_tensor("res_out", [H, TPC], F32, kind="ExternalOutput")
    moe_out_d = nc.dram_tensor("moe_out", [EPC, H, cap], BF16, kind="ExternalOutput")

    KG = 4  # contraction chunks per weight-stream DMA (tn=256 gemms)
    SW = 8  # m-chunks per sweep for TN=256 GEMMs (4 paired psum banks)

    with tile.TileContext(nc) as tc:
        with (
            tc.tile_pool(name="res", bufs=1) as res,
            tc.tile_pool(name="stream", bufs=3) as stream,
            tc.tile_pool(name="small", bufs=2) as small,
            tc.tile_pool(name="outp", bufs=2) as outp,
            tc.tile_pool(name="psum", bufs=4, space="PSUM") as psum,
            tc.tile_pool(name="dram", bufs=1, space="DRAM") as dram,
        ):
            eng_rr = [nc.sync, nc.scalar]

            # ---------------- early resident loads ----------------
            # xT first on sync (first qkv matmul needs chunk 0), cos/sin on
            # scalar; everything small goes on the gpsimd SWDGE ring so the
            # two HWDGE rings start streaming weights immediately.
            xT_sb = res.tile([BLK, KQ * TPC], BF16, tag="xT")
            nc.scalar.dma_start(xT_sb[:].rearrange("p (k t) -> p k t", k=KQ),
                               xT_bf_d.ap().rearrange("(k p) t -> p k t", p=BLK))
            cos2_sb = res.tile([HALF, 2 * TPC], F32, tag="cos")
            sin2_sb = res.tile([HALF, 2 * TPC], F32, tag="sin")
            scol_sb = res.tile([BLK, 2], F32, tag="scol")
            nc.gpsimd.dma_start(scol_sb[:], scol_d[:])
            oh_sb = res.tile([BLK, 64], BF16, tag="ohtab")
            nc.gpsimd.dma_start(oh_sb[:], oh_d[:])
            sel_sb = res.tile([8, 8 * BLK], BF16, tag="seltab")
            nc.gpsimd.dma_start(sel_sb[:], sel_d[:])
            bias_sb = res.tile([BLK, 2 * NBLK], F32, tag="bias")
            nc.gpsimd.dma_start(bias_sb[:], bias_d.ap().rearrange("a k p -> p (a k)"))
            tri4_sb = res.tile([BLK, GW], BF16, tag="tri4")
            nc.gpsimd.dma_start(tri4_sb[:], tri4_d[:])
            ident_sb = res.tile([BLK, BLK], BF16, tag="ident")
            nc.gpsimd.dma_start(ident_sb[:], ident_d[:])

            ones_row = res.tile([1, BLK], BF16, tag="onesr")
            nc.vector.memset(ones_row[:], 1.0)
            ones_cf = res.tile([BLK, 1], F32, tag="onescf")
            nc.vector.memset(ones_cf[:], 1.0)

            # PE warmup: the HAM clock gate holds the PE at 1.2GHz until it
            # has been busy ~3.4us; burn dummy matmuls while the first weight
            # chunks are still on the wire so the real sweeps start warm.
            wrm = res.tile([BLK, BLK], BF16, tag="wrm")
            nc.vector.memset(wrm[:], 0.001)
            for _w in range(40):
                wp = psum.tile([BLK, BLK], F32, tag="macct", bufs=2, name="wp")
                nc.tensor.matmul(wp[:], wrm[:], wrm[:], start=True, stop=True)

            ew_sb = res.tile([BLK, EPC * cap], F32, tag="ew")
            xg_sb = res.tile([BLK, EPC * KQ * cap], FP8, tag="xg")
            attnU_sb = res.tile([BLK, NH * TPC], BF16, tag="attnU")

            # q stored qb-major: col = qb*2048 + h*128 + t
            q_sb = res.tile([BLK, NH * TPC], BF16, tag="q")
            k_sb = res.tile([BLK, NKV * TPC], BF16, tag="k")
            v_sb = res.tile([BLK, 2 * NKV * HD], BF16, tag="v")
            resid_sb = res.tile([BLK, KQ * TPC], F32, tag="resid")
            h2_sb = res.tile([BLK, KQ * TPC], BF16, tag="h2")
            gu_sb = res.tile([BLK, KQ * TPC], BF16, tag="gu")
            hm_sb = res.tile([BLK, EPC * MI * cap], BF16, tag="hm")

            kag_sb = res.tile([BLK, NBLK * NKV * BLK], BF16, tag="kag")
            vag_sb = res.tile([BLK, NBLK * NKV * BLK], BF16, tag="vag")

            # ============ generic streamed GEMM sweep ============
            # dr=True: fp8 DoubleRow — weights fp8, rhs_fn(kp) returns a
            # [128, 2, tn] k-pair AP; two 128-row k-tiles per matmul.
            def gemm(w_src, mcnt, kcnt, rhs_fn, tag, tn, consume, sweep, kg, tag_bufs,
                     sweep_starts=None, engs=None, dr=False):
                engs = engs or eng_rr
                pair = 2 * tn <= 512
                for s0 in (sweep_starts if sweep_starts is not None
                           else range(0, mcnt, sweep)):
                    ms = list(range(s0, min(s0 + sweep, mcnt)))
                    mw = len(ms)
                    if pair:
                        nt = (mw + 1) // 2
                        pts = [psum.tile([BLK, 2 * tn], F32, tag=tag, bufs=tag_bufs,
                                         name=f"pt{j}") for j in range(nt)]
                        paps = [pts[j // 2][:, (j % 2) * tn:(j % 2 + 1) * tn]
                                for j in range(mw)]
                    else:
                        pts = [psum.tile([BLK, tn], F32, tag=tag, bufs=tag_bufs,
                                         name=f"pt{j}") for j in range(mw)]
                        paps = [pts[j][:] for j in range(mw)]
                    for kg0 in range(0, kcnt, kg):
                        kgn = min(kg, kcnt - kg0)
                        wt = stream.tile([BLK, KG * SW * BLK], FP8 if dr else BF16,
                                         tag="wt8" if dr else "wt")
                        engs[(kg0 // kg) % len(engs)].dma_start(
                            wt[:, :kgn * mw * BLK].rearrange("p (k c) -> p k c", k=kgn),
                            w_src(kg0, kgn, ms[0] * BLK, (ms[-1] + 1) * BLK))
                        wtv = wt[:, :kgn * mw * BLK].rearrange("p (k c) -> p k c", k=kgn)
                        for kl in range(0, kgn, 2 if dr else 1):
                            k = kg0 + kl
                            for j in range(mw):
                                first = (j % 2 == 0) if pair else True
                                last = (j % 2 == 1 or j == mw - 1) if pair else True
                                if dr:
                                    nc.tensor.matmul(
                                        paps[j],
                                        wtv[:, kl:kl + 2, j * BLK:(j + 1) * BLK],
                                        rhs_fn(k // 2), start=(k == 0 and first),
                                        stop=(k == kcnt - 2 and last), perf_mode=DR)
                                else:
                                    nc.tensor.matmul(
                                        paps[j], wt[:, (kl * mw + j) * BLK:(kl * mw + j + 1) * BLK],
                                        rhs_fn(k), start=(k == 0 and first),
                                        stop=(k == kcnt - 1 and last))
                    consume(ms, paps, pts)

            # ---------------- QKV projection (feature-major out) ----------------
            def rope_pair(pt2, dst_ap):
                # pt2: [128, 512] psum pair (two head-chunks side by side);
                # dst_ap(half) -> AP matching [64, 512] column order of pt2.
                t1 = small.tile([HALF, 2 * TPC], F32, tag="r1")
                t2 = small.tile([HALF, 2 * TPC], F32, tag="r2")
                nc.vector.tensor_mul(t1[:], pt2[0:HALF, :], cos2_sb[:])
                nc.vector.tensor_mul(t2[:], pt2[HALF:BLK, :], sin2_sb[:])
                nc.vector.tensor_sub(dst_ap(0), t1[:], t2[:])
                t3 = small.tile([HALF, 2 * TPC], F32, tag="r1")
                t4 = small.tile([HALF, 2 * TPC], F32, tag="r2")
                nc.vector.tensor_mul(t3[:], pt2[HALF:BLK, :], cos2_sb[:])
                nc.vector.tensor_mul(t4[:], pt2[0:HALF, :], sin2_sb[:])
                nc.vector.tensor_add(dst_ap(1), t3[:], t4[:])

            def q_dst(m, half):
                # heads m, m+1; psum col order (h, qb0 128 | qb1 128);
                # q_sb col = qb*2048 + h*128 + t
                lo = half * HALF
                return q_sb[lo:lo + HALF, :] \
                    .rearrange("p (qb h t) -> p h qb t", qb=2, h=NH)[:, m:m + 2]

            def qkv_consume(ms, paps, pts):
                for jt, pt2 in enumerate(pts):
                    m = ms[2 * jt]
                    if m < NH:
                        rope_pair(pt2[:], lambda half, m=m: q_dst(m, half))
                    elif m < NH + NKV:
                        kvh = m - NH
                        rope_pair(pt2[:], lambda half, kvh=kvh: k_sb[
                            half * HALF:(half + 1) * HALF,
                            kvh * TPC:(kvh + 2) * TPC])
                    else:
                        for half_j in range(2):
                            kvh = m + half_j - NH - NKV
                            ps = pt2[:, half_j * TPC:(half_j + 1) * TPC]
                            vtmp = small.tile([BLK, TPC], BF16, tag="vtmp")
                            nc.vector.tensor_copy(vtmp[:], ps)
                            for tb in range(2):
                                ptt = psum.tile([BLK, BLK], BF16, tag="macct",
                                                bufs=2, name="ptt")
                                nc.tensor.transpose(ptt[:], vtmp[:, tb * BLK:(tb + 1) * BLK], ident_sb[:])
                                nc.vector.tensor_scalar_mul(
                                    v_sb[:, (tb * NKV + kvh) * BLK:(tb * NKV + kvh + 1) * BLK],
                                    ptt[:], scol_sb[:, tb:tb + 1])

            wqkv_src = lambda kg0, kgn, c0, c1: wqkv_d[kg0 * BLK:(kg0 + kgn) * BLK, c0:c1] \
                .rearrange("(k p) c -> p k c", p=BLK)
            # KV chunks first so the AllGather can launch early. qkv weights
            # stream on the scalar ring; wsT gets the sync ring so MoE gu
            # is never DMA-starved into the attention phase.
            qkv_rhs = lambda k: xT_sb[:, k * TPC:(k + 1) * TPC]
            for _rep in range(2):
                nc.scalar.dma_start(cos2_sb[:, _rep * TPC:(_rep + 1) * TPC], cos_d[:])
                nc.scalar.dma_start(sin2_sb[:, _rep * TPC:(_rep + 1) * TPC], sin_d[:])
            gemm(wqkv_src, NH + 2 * NKV, KQ, qkv_rhs, "acct", TPC, qkv_consume,
                 SW, KG, 4, sweep_starts=[16], engs=[nc.sync])

            # ---------------- KV AllGather (split shallow/deep) ----------------
            # Two collectives: each core's shallow block (kb 0..7 = even AG
            # slots) first, deep block second — the qb0 attention passes need
            # only the shallow half, so they can start while the deep half is
            # still on the wire.
            KSZ2 = NKV * BLK * BLK  # 65536 elems per half
            kv_sh = dram.tile([2, KSZ2], BF16)
            kv_dp = dram.tile([2, KSZ2], BF16)
            ag_sh = dram.tile([NC_, 2, KSZ2], BF16, addr_space="Shared")
            ag_dp = dram.tile([NC_, 2, KSZ2], BF16, addr_space="Shared")
            kv_loc = [kv_sh, kv_dp]
            for half in range(2):
                nc.gpsimd.dma_start(
                    kv_loc[half][0, :].rearrange("(h d t) -> d h t", h=NKV, d=BLK),
                    k_sb[:].rearrange("d (h t) -> d h t", h=NKV)[:, :, half * BLK:(half + 1) * BLK])
                nc.gpsimd.dma_start(
                    kv_loc[half][1, :].rearrange("(h t d) -> t h d", h=NKV, t=BLK),
                    v_sb[:].rearrange("t (b h d) -> b t h d", b=2, h=NKV)[half])
            nc.gpsimd.collective_compute(
                "AllGather", mybir.AluOpType.bypass,
                replica_groups=[list(range(NC_))],
                ins=[kv_sh[:]], outs=[ag_sh[:]])
            nc.gpsimd.collective_compute(
                "AllGather", mybir.AluOpType.bypass,
                replica_groups=[list(range(NC_))],
                ins=[kv_dp[:]], outs=[ag_dp[:]])
            # remaining qkv sweeps (q heads) overlap the collective
            gemm(wqkv_src, NH + 2 * NKV, KQ, qkv_rhs, "acct", TPC, qkv_consume,
                 SW, KG, 4, sweep_starts=[0, 8], engs=[nc.scalar])
            # deferred resident loads (needed by MoE / wo, not by qkv)
            for e in range(EPC):
                nc.scalar.dma_start(
                    xg_sb[:, e * KQ * cap:(e + 1) * KQ * cap].rearrange("p (k t) -> p k t", k=KQ),
                    xg_d[e].rearrange("(k p) t -> p k t", p=BLK))
                nc.scalar.dma_start(ew_sb[:, e * cap:(e + 1) * cap], ew_d[e])

            # ------------------- MoE sweeps (as thunks) -------------------
            def make_gu_thunks(e):
                thunks = []

                def gu_consume(ms, paps, pts, e=e):
                    for j in range(0, len(ms), 2):
                        p = (ms[0] + j) // 2
                        sg = small.tile([BLK, cap], BF16, tag="sg")
                        # psums are 32x (fp8 weight scale); silu(g) needs
                        # scale 1/32; the 32x on u is folded into ew.
                        nc.scalar.activation(sg[:], paps[j], AF.Silu, scale=1.0 / WS8)
                        nc.vector.tensor_mul(
                            hm_sb[:, (e * MI + p) * cap:(e * MI + p + 1) * cap],
                            sg[:], paps[j + 1])

                ws_src = lambda kg0, kgn, c0, c1, e=e: wsT_d[e, kg0 * BLK:(kg0 + kgn) * BLK, c0:c1] \
                    .rearrange("(k p) c -> p k c", p=BLK)
                xgv = xg_sb[:, e * KQ * cap:(e + 1) * KQ * cap] \
                    .rearrange("p (k t) -> p k t", k=KQ)
                rhs_e = lambda kp, xgv=xgv: xgv[:, 2 * kp:2 * kp + 2, :]
                for s0 in range(0, 2 * MI, 2):
                    thunks.append(lambda s0=s0, f=gu_consume, w=ws_src, r=rhs_e: gemm(
                        w, 2 * MI, KQ, r, "macct", cap, f, 2, 8, 2, sweep_starts=[s0],
                        engs=[nc.sync], dr=True))
                return thunks

            def make_w2s_thunks(e, tag, bufs, sweep):
                thunks = []

                def w2s_consume(ms, paps, pts, e=e):
                    for m, ps in zip(ms, paps):
                        mo = outp.tile([BLK, cap], BF16, tag="mo")
                        nc.vector.tensor_mul(mo[:], ps, ew_sb[:, e * cap:(e + 1) * cap])
                        eng_rr[m % 2].dma_start(moe_out_d[e, m * BLK:(m + 1) * BLK, :], mo[:])

                w2s_src = lambda kg0, kgn, c0, c1, e=e: w2sT_d[e, kg0 * BLK:(kg0 + kgn) * BLK, c0:c1] \
                    .rearrange("(k p) c -> p k c", p=BLK)
                for s0 in range(0, KQ, sweep):
                    thunks.append(lambda s0=s0, e=e, f=w2s_consume, w=w2s_src: gemm(
                        w, KQ, MI,
                        lambda k, e=e: hm_sb[:, (e * MI + k) * cap:(e * MI + k + 1) * cap],
                        tag, cap, f, sweep, 8, bufs, sweep_starts=[s0]))
                return thunks

            # both experts' gu inline before attention (silu done before any
            # exp act -> no act-table churn inside the attention phase)
            for th in make_gu_thunks(0):
                th()
            for th in make_gu_thunks(1):
                th()

            # unpack AG results on the gpsimd SWDGE ring (doesn't block the
            # HWDGE weight streams). slot = 2c+sub; attention maps kb->slot.
            ags = [ag_sh, ag_dp]
            for sub in range(2):
                for hh in range(NKV):
                    nc.gpsimd.dma_start(
                        kag_sb[:].rearrange("d (c s h t) -> s h d c t", c=NC_, s=2, h=NKV)[sub, hh],
                        ags[sub][:, 0, :].rearrange("c (h d t) -> h d c t", h=NKV, d=BLK)[hh])
                    nc.gpsimd.dma_start(
                        vag_sb[:].rearrange("t (c s h dd) -> s h t c dd", c=NC_, s=2, h=NKV)[sub, hh],
                        ags[sub][:, 1, :].rearrange("c (h t dd) -> h t c dd", h=NKV, t=BLK)[hh])

            # interleave thunks: w2s for both experts across attention
            # (consumes are vector-only -> scalar engine stays exp-only)
            fill = make_w2s_thunks(0, "macct", 2, 2) + make_w2s_thunks(1, "macct", 2, 2)

            # ---------------- attention: 4 kv-groups x 2 qb passes ----------------
            # pass (g, qb): 4 heads 4g..4g+3, queries = qb block (128 tokens).
            # dense kbs: qb0 -> kb 0..7, qb1 -> kb 0..15; diag = own block.
            # one pinned psum bank accumulates all 8 passes' denominators,
            # pass p landing on partition row p (one-hot stationary), so a
            # single [8,512] reciprocal serves the whole attention phase.
            dps = psum.tile([BLK, GW], F32, tag="dps", bufs=1)

            def attn_pass(g, qb, p):
                kbs = list(range(NC_ if qb == 0 else NBLK))
                nkb = len(kbs) + 1  # + diag
                qv = q_sb[:, qb * NH * BLK + 4 * g * BLK: qb * NH * BLK + 4 * (g + 1) * BLK]
                aps = psum.tile([BLK, GW], F32, tag="aps", bufs=1)
                for idx in range(nkb):
                    diag = idx == nkb - 1
                    sc = psum.tile([BLK, GW], F32, tag="acct", bufs=4, name="sc")
                    if diag:
                        kap = k_sb[:, g * TPC + qb * BLK: g * TPC + (qb + 1) * BLK]
                        vap = v_sb[:, (qb * NKV + g) * BLK:(qb * NKV + g + 1) * BLK]
                    else:
                        sl = _slot(kbs[idx])
                        kap = kag_sb[:, (sl * NKV + g) * BLK:(sl * NKV + g + 1) * BLK]
                        vap = vag_sb[:, (sl * NKV + g) * BLK:(sl * NKV + g + 1) * BLK]
                    nc.tensor.matmul(sc[:], kap, qv, start=True, stop=True)
                    pdn = small.tile([BLK, GW], BF16, tag="pdn", bufs=6)
                    if diag:
                        nc.scalar.activation(pdn[:], sc[:], AF.Exp, scale=SCALE)
                        nc.vector.tensor_mul(pdn[:], pdn[:], tri4_sb[:])
                    else:
                        nc.scalar.activation(pdn[:], sc[:], AF.Exp, scale=SCALE,
                                             bias=bias_sb[:, qb * NBLK + kbs[idx]:
                                                          qb * NBLK + kbs[idx] + 1])
                    nc.tensor.matmul(aps[:], vap, pdn[:], start=(idx == 0),
                                     stop=(idx == nkb - 1))
                    nc.tensor.matmul(dps[0:8, :], oh_sb[:, 8 * p:8 * p + 8], pdn[:],
                                     start=(p == 0 and idx == 0),
                                     stop=(p == 7 and idx == nkb - 1))
                # drain unnormalized; normalize batched after all passes
                nc.vector.tensor_copy(
                    attnU_sb[:, qb * NH * BLK + 4 * g * BLK: qb * NH * BLK + 4 * (g + 1) * BLK],
                    aps[:])

            # shallow (qb0) passes first — they only need ag_sh — then deep;
            # 12 w2s thunks inside attention, 4 left for the wo ramp.
            fi = 0
            for g in range(4):
                fill[fi](); fi += 1
                attn_pass(g, 0, g)
            for g in range(4):
                fill[fi](); fi += 1
                fill[fi](); fi += 1
                attn_pass(g, 1, 4 + g)

            # ---------------- batched softmax normalization ----------------
            rec_all = small.tile([8, GW], BF16, tag="rec")
            with nc.allow_low_precision(reason="softmax denom recip, ~2^-9 rel"):
                nc.vector.reciprocal(rec_all[:], dps[0:8, :])
            for p in range(8):
                g, qb = (p, 0) if p < 4 else (p - 4, 1)
                bcp = psum.tile([BLK, GW], F32, tag="acct", bufs=4, name="bcp")
                nc.tensor.matmul(bcp[:], sel_sb[:, p * BLK:(p + 1) * BLK],
                                 rec_all[:], start=True, stop=True)
                bcs = small.tile([BLK, GW], F32, tag="bcs")
                nc.vector.tensor_copy(bcs[:], bcp[:])
                au = attnU_sb[:, qb * NH * BLK + 4 * g * BLK:
                              qb * NH * BLK + 4 * (g + 1) * BLK]
                nc.vector.tensor_mul(au, au, bcs[:])

            # ---------------- wo + residual ----------------
            def wo_consume(ms, paps, pts):
                for jt, pt in enumerate(pts):
                    m0 = ms[2 * jt]
                    w = pt.shape[1]
                    nc.vector.tensor_add(resid_sb[:, m0 * TPC: m0 * TPC + w],
                                         pt[:], xT_sb[:, m0 * TPC: m0 * TPC + w])

            wo_src = lambda kg0, kgn, c0, c1: wo_d[kg0 * BLK:(kg0 + kgn) * BLK, c0:c1] \
                .rearrange("(k p) c -> p k c", p=BLK)
            wo_rhs = lambda k: attnU_sb[:].rearrange(
                "p (qb h t) -> p h qb t", qb=2, h=NH)[:, k]
            for i, s0 in enumerate(range(0, KQ, SW)):
                if fi < len(fill):
                    fill[fi](); fi += 1
                gemm(wo_src, KQ, KQ, wo_rhs, "acct", TPC, wo_consume, SW, KG, 4,
                     sweep_starts=[s0])
                if fi < len(fill):
                    fill[fi](); fi += 1
            while fi < len(fill):
                fill[fi](); fi += 1

            # ---------------- residual MLP norm scale ----------------
            ssq = psum.tile([BLK, TPC], F32, tag="macct", bufs=2, name="ssq")
            for k in range(KQ):
                sq = small.tile([BLK, TPC], F32, tag="sq")
                nc.vector.tensor_mul(sq[:], resid_sb[:, k * TPC:(k + 1) * TPC],
                                     resid_sb[:, k * TPC:(k + 1) * TPC])
                nc.tensor.matmul(ssq[0:1, :], ones_cf[:], sq[:],
                                 start=(k == 0), stop=(k == KQ - 1))
            vtmp2 = small.tile([1, TPC], F32, tag="vt")
            nc.vector.tensor_scalar(vtmp2[:], ssq[0:1, :], 1.0 / H, EPS,
                                    mybir.AluOpType.mult, mybir.AluOpType.add)
            st = small.tile([1, TPC], F32, tag="vt2")
            nc.scalar.activation(st[:], vtmp2[:], AF.Sqrt)
            s2r = small.tile([1, TPC], BF16, tag="vt3")
            with nc.allow_low_precision(reason="rms-norm scale recip, ~2^-9 rel"):
                nc.vector.reciprocal(s2r[:], st[:])
            s2p = psum.tile([BLK, TPC], F32, tag="macct", bufs=2, name="s2p")
            nc.tensor.matmul(s2p[:], ones_row[:], s2r[:], start=True, stop=True)
            s2s = small.tile([BLK, TPC], F32, tag="s2s")
            nc.scalar.activation(s2s[:], s2p[:], AF.Copy)
            for k in range(KQ):
                nc.vector.tensor_mul(h2_sb[:, k * TPC:(k + 1) * TPC],
                                     resid_sb[:, k * TPC:(k + 1) * TPC], s2s[:])

            # ---------------- w13 (interleaved g/u) + silu_and_mul ----------------
            def w13_consume(ms, paps, pts):
                for jt, pt in enumerate(pts):
                    p = ms[2 * jt] // 2
                    sg = small.tile([BLK, TPC], BF16, tag="sg13")
                    nc.scalar.activation(sg[:], pt[:, 0:TPC], AF.Silu)
                    nc.vector.tensor_mul(gu_sb[:, p * TPC:(p + 1) * TPC],
                                         sg[:], pt[:, TPC:2 * TPC])

            w13_src = lambda kg0, kgn, c0, c1: w13_d[kg0 * BLK:(kg0 + kgn) * BLK, c0:c1] \
                .rearrange("(k p) c -> p k c", p=BLK)
            gemm(w13_src, 2 * KQ, KQ, lambda k: h2_sb[:, k * TPC:(k + 1) * TPC],
                 "acct", TPC, w13_consume, SW, KG, 4)

            # ---------------- w2 + final out ----------------
            def w2_consume(ms, paps, pts):
                for jt, pt in enumerate(pts):
                    m0 = ms[2 * jt]
                    w = pt.shape[1]
                    fo = outp.tile([BLK, 2 * TPC], F32, tag="fo")
                    nc.vector.tensor_add(fo[:, :w], pt[:], resid_sb[:, m0 * TPC:m0 * TPC + w])
                    nc.sync.dma_start(
                        res_out_d.ap().rearrange("(m p) t -> p m t", p=BLK)[:, m0:m0 + w // TPC],
                        fo[:, :w].rearrange("p (m t) -> p m t", t=TPC))

            w2_src = lambda kg0, kgn, c0, c1: w2_d[kg0 * BLK:(kg0 + kgn) * BLK, c0:c1] \
                .rearrange("(k p) c -> p k c", p=BLK)
            gemm(w2_src, KQ, KQ, lambda k: gu_sb[:, k * TPC:(k + 1) * TPC],
                 "acct", TPC, w2_consume, SW, KG, 4)

    nc.compile()
    return nc


def _interleave_cols(w, half):
    # [rows, 2*half] -> column chunks reordered so chunk 2p=g_p, 2p+1=u_p
    rows = w.shape[0]
    g = w[:, :half].reshape(rows, half // BLK, BLK)
    u = w[:, half:].reshape(rows, half // BLK, BLK)
    out = np.empty((rows, 2 * (half // BLK), BLK), w.dtype)
    out[:, 0::2] = g
    out[:, 1::2] = u
    return out.reshape(rows, 2 * half // BLK * BLK)


def kernel(**inputs):
    global LAST_RESULT
    hidden = f32(inputs["hidden_states"])
    positions = np.asarray(inputs["positions"]).astype(np.float32)
    ln_in_w = f32(inputs["ln_in_w"])
    ln_post_w = f32(inputs["ln_post_w"])
    ln_res_w = f32(inputs["ln_res_w"])
    wqkv = f32(inputs["wqkv"])
    wo = f32(inputs["wo"])
    res_w13 = f32(inputs["res_w13"])
    res_w2 = f32(inputs["res_w2"])
    gate_w = f32(inputs["gate_w"])
    ws = f32(inputs["ws"])
    w2s = f32(inputs["w2s"])

    # ---- host prep (sharding) ----
    s = 1.0 / np.sqrt(np.mean(hidden * hidden, axis=1) + EPS)  # [T]
    x_norm = hidden * s[:, None]

    logits = (x_norm * ln_post_w) @ gate_w
    pr = np.exp(logits - logits.max(-1, keepdims=True))
    pr /= pr.sum(-1, keepdims=True)
    topi = np.argsort(-pr, axis=-1, kind="stable")[:, :TOPK]
    topw = np.take_along_axis(pr, topi, axis=-1)
    topw /= topw.sum(-1, keepdims=True)
    tok_lists = [np.where((topi == e).any(-1))[0] for e in range(E)]
    wts = [np.sum(np.where(topi[tl] == e, topw[tl], 0.0), -1).astype(np.float32)
           for e, tl in zip(range(E), tok_lists)]
    cap = max(128, -(-max(len(t) for t in tok_lists) // 32) * 32)
    assert cap <= 512, cap

    ck = cap
    if ck not in _CACHE:
        _CACHE[ck] = _build(cap)
    nc = _CACHE[ck]

    inv_freq = 1.0 / (THETA ** (np.arange(0, HD, 2, dtype=np.float32) / HD))
    ang = positions[:, None] * inv_freq
    cos_t, sin_t = np.cos(ang), np.sin(ang)

    tri01 = (np.arange(BLK)[None, :] >= np.arange(BLK)[:, None]).astype(np.float32)
    tri4 = np.tile(tri01, (1, 4))
    ident = np.eye(BLK, dtype=np.float32)
    ohtab = np.zeros((BLK, 64), np.float32)
    for p in range(8):
        ohtab[:, p * 8 + p] = 1.0
    seltab = np.zeros((8, 8 * BLK), np.float32)
    for p in range(8):
        seltab[p, p * BLK:(p + 1) * BLK] = 1.0

    wqkv_f = wqkv * ln_in_w[:, None]
    w13_f = _interleave_cols(res_w13 * ln_res_w[:, None], H)
    x_norm_post = x_norm * ln_post_w
    wsT = ws.transpose(0, 2, 1)  # [E, H, 2I]
    wsT_il = np.stack([_interleave_cols(wsT[e], I) for e in range(E)])
    w2sT = w2s.transpose(0, 2, 1)

    shared = {
        "tri4": bf(tri4), "ident": bf(ident),
        "ohtab": bf(ohtab), "seltab": bf(seltab),
        "wqkv": bf(wqkv_f), "wo": bf(wo), "w13": bf(w13_f), "w2": bf(res_w2),
    }

    in_maps = []
    own = [[i, NBLK - 1 - i] for i in range(NC_)]
    for i in range(NC_):
        toks = np.concatenate([np.arange(b * BLK, (b + 1) * BLK) for b in own[i]])
        xT = hidden[toks].T
        cs = (cos_t[toks] * s[toks, None]).T
        sn = (sin_t[toks] * s[toks, None]).T
        scol = np.stack([s[toks[:BLK]], s[toks[BLK:]]], axis=1)
        bias = np.zeros((2, NBLK, BLK), np.float32)
        b0, b1 = own[i]
        bias[0, b0:, :] = NEG
        bias[1, b1:, :] = NEG
        exps = [2 * i, 2 * i + 1]
        xg = np.zeros((EPC, H, cap), np.float32)
        ew = np.zeros((EPC, BLK, cap), np.float32)
        for j, e in enumerate(exps):
            n = len(tok_lists[e])
            xg[j, :, :n] = x_norm_post[tok_lists[e]].T
            # ws runs in fp8 with weights x32; u-path 32x folded here
            ew[j, :, :n] = wts[e][None, :] / WS8
        in_maps.append({
            "xT_bf": bf(xT),
            "cos_s": f32(cs), "sin_s": f32(sn), "s_col": f32(scol),
            "bias": bias,
            "wsT": f8(wsT_il[exps] * WS8),
            "w2sT": bf(w2sT[exps]),
            "xgT": f8(xg), "ew": ew,
            **shared,
        })

    res = run_bass_kernel_spmd(nc, in_maps, core_ids=list(range(NC_)), trace=TRACE)
    LAST_RESULT = res

    out = np.zeros((T, H), np.float32)
    for i in range(NC_):
        toks = np.concatenate([np.arange(b * BLK, (b + 1) * BLK) for b in own[i]])
        out[toks] = res.results[i]["res_out"].T
    for i in range(NC_):
        for j, e in enumerate((2 * i, 2 * i + 1)):
            tl = tok_lists[e]
            out[tl] += res.results[i]["moe_out"][j].astype(np.float32).T[:len(tl)]
    return out


# revision 78
# speedup vs baseline: 1.0554x; 1.0289x over previous
"""ArcticDecoderLayer on 8 TRN2 NeuronCores.

Sharding:
  - tokens: zigzag block-parallel (core i owns 128-token blocks {i, 15-i});
    attention, wo, residual MLP are token-parallel (weights replicated).
  - MoE: expert-parallel, 2 experts/core; routing/top-2 + token gather/scatter
    done host-side (part of shard/unshard), expert GEMMs on device.
  - One AllGather (K^T feature-major + V token-major, bf16) is the only
    collective; causal masking is data-driven (per-core exp-bias columns) so
    the SPMD graph is identical on all cores.

Restructure history (trace-driven, from the 800us v1 baseline):
  - Attention processes 4 GQA heads per matmul (shared K/V stationary,
    512-col streams) in 2 passes per kv-group (shallow qb0: 8 kbs+diag,
    deep qb1: 16 kbs+diag). Queries stored qb-major so causal masks stay
    per-partition exp biases; diag tri mask = exp * tri01 on vector.
  - Softmax denominators: every pass accumulates into its own partition row
    of ONE pinned psum bank (one-hot stationary), so a single [8,512]
    reciprocal + selector-matrix broadcasts normalize everything (a [1,N]
    reciprocal runs on a single DVE lane at ~7.6ns/elem - avoid).
  - MoE expert weights + expert inputs in fp8e4 (x32 pow-2 scale, dequant
    folded into the silu scale arg + ew), DoubleRow matmuls: [p,2,cols]
    k-pair APs on both operands. qkv/wo/w13/w2 stay bf16 (fp8 there costs
    1.4-1.7e-2 max-rel vs the 2e-2 gate; ws8+xg8 costs 1.15e-2).
  - KV AllGather split into shallow-half and deep-half collectives; qb0
    passes only need the shallow half. Packs/unpacks + small residents ride
    the gpsimd SWDGE ring so they never head-of-line-block the two HWDGE
    weight rings (sync=wsT/gu, scalar=xT/wqkv/xg).
  - Scalar engine stays exp-only inside attention (all silu before, table
    reloads cost 1.3us); attention PE filled with w2s thunks (vector-only
    consumes).
  - PSUM: "acct" (4 banks) gemm sweeps + score tiles + bcp; "aps" drain +
    "dps" den pin (2); "macct" (2) MoE thunks = 8 exactly.
"""
import numpy as np
import ml_dtypes

import concourse.bacc as bacc
import concourse.tile as tile
import concourse.mybir as mybir
from concourse.bass_utils import run_bass_kernel_spmd

F32 = mybir.dt.float32
BF16 = mybir.dt.bfloat16
FP8 = mybir.dt.float8e4
DR = mybir.MatmulPerfMode.DoubleRow
AF = mybir.ActivationFunctionType
WS8 = 32.0  # power-of-2 fp8 weight scale (dequant folded host-side)

H = 2048
NH = 16
NKV = 4
HD = 128
HALF = 64
I = 1024
E = 16
TOPK = 2
T = 2048
EPS = 1e-5
THETA = 10000.0
NC_ = 8
BLK = 128
NBLK = 16
TPC = 256  # tokens per core
EPC = 2  # experts per core
SCALE = HD ** -0.5
NEG = -30000.0
KQ = H // BLK  # 16
MI = I // BLK  # 8
GW = 4 * BLK  # 512 = one attention group-pass stream width (4 heads x 128)

TRACE = False
LAST_RESULT = None
_CACHE = {}

bf = lambda a: np.ascontiguousarray(np.asarray(a).astype(ml_dtypes.bfloat16))
f32 = lambda a: np.ascontiguousarray(a, dtype=np.float32)
f8 = lambda a: np.ascontiguousarray(np.asarray(a).astype(ml_dtypes.float8_e4m3fn))


def _slot(kb):
    c = min(kb, NBLK - 1 - kb)
    return 2 * c + (0 if kb < NC_ else 1)


def _build(cap):
    nc = bacc.Bacc("TRN2", target_bir_lowering=False, debug=False, num_devices=NC_)

    din = lambda name, shape, dt=BF16: nc.dram_tensor(name, shape, dt, kind="ExternalInput")
    xT_bf_d = din("xT_bf", [H, TPC])
    cos_d = din("cos_s", [HALF, TPC], F32)
    sin_d = din("sin_s", [HALF, TPC], F32)
    scol_d = din("s_col", [BLK, 2], F32)
    oh_d = din("ohtab", [BLK, 64])   # col p*8+j = (j==p): den -> psum row p
    sel_d = din("seltab", [8, 8 * BLK])  # [j, p*128+r] = (j==p): row-p bcast
    bias_d = din("bias", [2, NBLK, BLK], F32)
    tri4_d = din("tri4", [BLK, GW])  # bf16 0/1 causal mask tiled x4 heads
    ident_d = din("ident", [BLK, BLK])
    wqkv_d = din("wqkv", [H, NH * HD + 2 * NKV * HD])
    wo_d = din("wo", [NH * HD, H])
    w13_d = din("w13", [H, 2 * H])  # host-interleaved: chunk 2p=g_p, 2p+1=u_p
    w2_d = din("w2", [H, H])
    wsT_d = din("wsT", [EPC, H, 2 * I], FP8)  # host-interleaved g/u pairs
    w2sT_d = din("w2sT", [EPC, I, H])
    xg_d = din("xgT", [EPC, H, cap], FP8)
    ew_d = din("ew", [EPC, BLK, cap], F32)

    res_out_d = nc.dram_tensor("res_out", [H, TPC], F32, kind="ExternalOutput")
    moe_out_d = nc.dram_tensor("moe_out", [EPC, H, cap], BF16, kind="ExternalOutput")

    KG = 4  # contraction chunks per weight-stream DMA (tn=256 gemms)
    SW = 8  # m-chunks per sweep for TN=256 GEMMs (4 paired psum banks)

    with tile.TileContext(nc) as tc:
        with (
            tc.tile_pool(name="res", bufs=1) as res,
            tc.tile_pool(name="stream", bufs=3) as stream,
            tc.tile_pool(name="small", bufs=2) as small,
            tc.tile_pool(name="outp", bufs=2) as outp,
            tc.tile_pool(name="psum", bufs=4, space="PSUM") as psum,
            tc.tile_pool(name="dram", bufs=1, space="DRAM") as dram,
        ):
            eng_rr = [nc.sync, nc.scalar]

            # ---------------- early resident loads ----------------
            # xT first on sync (first qkv matmul needs chunk 0), cos/sin on
            # scalar; everything small goes on the gpsimd SWDGE ring so the
            # two HWDGE rings start streaming weights immediately.
            xT_sb = res.tile([BLK, KQ * TPC], BF16, tag="xT")
            nc.scalar.dma_start(xT_sb[:].rearrange("p (k t) -> p k t", k=KQ),
                               xT_bf_d.ap().rearrange("(k p) t -> p k t", p=BLK))
            cos2_sb = res.tile([HALF, 2 * TPC], F32, tag="cos")
            sin2_sb = res.tile([HALF, 2 * TPC], F32, tag="sin")
            scol_sb = res.tile([BLK, 2], F32, tag="scol")
            nc.gpsimd.dma_start(scol_sb[:], scol_d[:])
            oh_sb = res.tile([BLK, 64], BF16, tag="ohtab")
            nc.gpsimd.dma_start(oh_sb[:], oh_d[:])
            sel_sb = res.tile([8, 8 * BLK], BF16, tag="seltab")
            nc.gpsimd.dma_start(sel_sb[:], sel_d[:])
            bias_sb = res.tile([BLK, 2 * NBLK], F32, tag="bias")
            nc.gpsimd.dma_start(bias_sb[:], bias_d.ap().rearrange("a k p -> p (a k)"))
            tri4_sb = res.tile([BLK, GW], BF16, tag="tri4")
            nc.gpsimd.dma_start(tri4_sb[:], tri4_d[:])
            ident_sb = res.tile([BLK, BLK], BF16, tag="ident")
            nc.gpsimd.dma_start(ident_sb[:], ident_d[:])

            ones_row = res.tile([1, BLK], BF16, tag="onesr")
            nc.vector.memset(ones_row[:], 1.0)
            ones_cf = res.tile([BLK, 1], F32, tag="onescf")
            nc.vector.memset(ones_cf[:], 1.0)

            # PE warmup: the HAM clock gate holds the PE at 1.2GHz until it
            # has been busy ~3.4us; burn dummy matmuls while the first weight
            # chunks are still on the wire so the real sweeps start warm.
            wrm = res.tile([BLK, BLK], BF16, tag="wrm")
            nc.vector.memset(wrm[:], 0.001)
            for _w in range(40):
                wp = psum.tile([BLK, BLK], F32, tag="macct", bufs=2, name="wp")
                nc.tensor.matmul(wp[:], wrm[:], wrm[:], start=True, stop=True)

            ew_sb = res.tile([BLK, EPC * cap], F32, tag="ew")
            xg_sb = res.tile([BLK, EPC * KQ * cap], FP8, tag="xg")
            attnU_sb = res.tile([BLK, NH * TPC], BF16, tag="attnU")

            # q stored qb-major: col = qb*2048 + h*128 + t
            q_sb = res.tile([BLK, NH * TPC], BF16, tag="q")
            k_sb = res.tile([BLK, NKV * TPC], BF16, tag="k")
            v_sb = res.tile([BLK, 2 * NKV * HD], BF16, tag="v")
            resid_sb = res.tile([BLK, KQ * TPC], F32, tag="resid")
            h2_sb = res.tile([BLK, KQ * TPC], BF16, tag="h2")
            gu_sb = res.tile([BLK, KQ * TPC], BF16, tag="gu")
            hm_sb = res.tile([BLK, EPC * MI * cap], BF16, tag="hm")

            kag_sb = res.tile([BLK, NBLK * NKV * BLK], BF16, tag="kag")
            vag_sb = res.tile([BLK, NBLK * NKV * BLK], BF16, tag="vag")

            # ============ generic streamed GEMM sweep ============
            # dr=True: fp8 DoubleRow — weights fp8, rhs_fn(kp) returns a
            # [128, 2, tn] k-pair AP; two 128-row k-tiles per matmul.
            def gemm(w_src, mcnt, kcnt, rhs_fn, tag, tn, consume, sweep, kg, tag_bufs,
                     sweep_starts=None, engs=None, dr=False):
                engs = engs or eng_rr
                pair = 2 * tn <= 512
                for s0 in (sweep_starts if sweep_starts is not None
                           else range(0, mcnt, sweep)):
                    ms = list(range(s0, min(s0 + sweep, mcnt)))
                    mw = len(ms)
                    if pair:
                        nt = (mw + 1) // 2
                        pts = [psum.tile([BLK, 2 * tn], F32, tag=tag, bufs=tag_bufs,
                                         name=f"pt{j}") for j in range(nt)]
                        paps = [pts[j // 2][:, (j % 2) * tn:(j % 2 + 1) * tn]
                                for j in range(mw)]
                    else:
                        pts = [psum.tile([BLK, tn], F32, tag=tag, bufs=tag_bufs,
                                         name=f"pt{j}") for j in range(mw)]
                        paps = [pts[j][:] for j in range(mw)]
                    for kg0 in range(0, kcnt, kg):
                        kgn = min(kg, kcnt - kg0)
                        wt = stream.tile([BLK, KG * SW * BLK], FP8 if dr else BF16,
                                         tag="wt8" if dr else "wt")
                        engs[(kg0 // kg) % len(engs)].dma_start(
                            wt[:, :kgn * mw * BLK].rearrange("p (k c) -> p k c", k=kgn),
                            w_src(kg0, kgn, ms[0] * BLK, (ms[-1] + 1) * BLK))
                        wtv = wt[:, :kgn * mw * BLK].rearrange("p (k c) -> p k c", k=kgn)
                        for kl in range(0, kgn, 2 if dr else 1):
                            k = kg0 + kl
                            for j in range(mw):
                                first = (j % 2 == 0) if pair else True
                                last = (j % 2 == 1 or j == mw - 1) if pair else True
                                if dr:
                                    nc.tensor.matmul(
                                        paps[j],
                                        wtv[:, kl:kl + 2, j * BLK:(j + 1) * BLK],
                                        rhs_fn(k // 2), start=(k == 0 and first),
                                        stop=(k == kcnt - 2 and last), perf_mode=DR)
                                else:
                                    nc.tensor.matmul(
                                        paps[j], wt[:, (kl * mw + j) * BLK:(kl * mw + j + 1) * BLK],
                                        rhs_fn(k), start=(k == 0 and first),
                                        stop=(k == kcnt - 1 and last))
                    consume(ms, paps, pts)

            # ---------------- QKV projection (feature-major out) ----------------
            def rope_pair(pt2, dst_ap):
                # pt2: [128, 512] psum pair (two head-chunks side by side);
                # dst_ap(half) -> AP matching [64, 512] column order of pt2.
                t1 = small.tile([HALF, 2 * TPC], F32, tag="r1")
                t2 = small.tile([HALF, 2 * TPC], F32, tag="r2")
                nc.vector.tensor_mul(t1[:], pt2[0:HALF, :], cos2_sb[:])
                nc.vector.tensor_mul(t2[:], pt2[HALF:BLK, :], sin2_sb[:])
                nc.vector.tensor_sub(dst_ap(0), t1[:], t2[:])
                t3 = small.tile([HALF, 2 * TPC], F32, tag="r1")
                t4 = small.tile([HALF, 2 * TPC], F32, tag="r2")
                nc.vector.tensor_mul(t3[:], pt2[HALF:BLK, :], cos2_sb[:])
                nc.vector.tensor_mul(t4[:], pt2[0:HALF, :], sin2_sb[:])
                nc.vector.tensor_add(dst_ap(1), t3[:], t4[:])

            def q_dst(m, half):
                # heads m, m+1; psum col order (h, qb0 128 | qb1 128);
                # q_sb col = qb*2048 + h*128 + t
                lo = half * HALF
                return q_sb[lo:lo + HALF, :] \
                    .rearrange("p (qb h t) -> p h qb t", qb=2, h=NH)[:, m:m + 2]

            def qkv_consume(ms, paps, pts):
                for jt, pt2 in enumerate(pts):
                    m = ms[2 * jt]
                    if m < NH:
                        rope_pair(pt2[:], lambda half, m=m: q_dst(m, half))
                    elif m < NH + NKV:
                        kvh = m - NH
                        rope_pair(pt2[:], lambda half, kvh=kvh: k_sb[
                            half * HALF:(half + 1) * HALF,
                            kvh * TPC:(kvh + 2) * TPC])
                    else:
                        for half_j in range(2):
                            kvh = m + half_j - NH - NKV
                            ps = pt2[:, half_j * TPC:(half_j + 1) * TPC]
                            vtmp = small.tile([BLK, TPC], BF16, tag="vtmp")
                            nc.vector.tensor_copy(vtmp[:], ps)
                            for tb in range(2):
                                ptt = psum.tile([BLK, BLK], BF16, tag="macct",
                                                bufs=2, name="ptt")
                                nc.tensor.transpose(ptt[:], vtmp[:, tb * BLK:(tb + 1) * BLK], ident_sb[:])
                                nc.vector.tensor_scalar_mul(
                                    v_sb[:, (tb * NKV + kvh) * BLK:(tb * NKV + kvh + 1) * BLK],
                                    ptt[:], scol_sb[:, tb:tb + 1])

            wqkv_src = lambda kg0, kgn, c0, c1: wqkv_d[kg0 * BLK:(kg0 + kgn) * BLK, c0:c1] \
                .rearrange("(k p) c -> p k c", p=BLK)
            # KV chunks first so the AllGather can launch early. qkv weights
            # stream on the scalar ring; wsT gets the sync ring so MoE gu
            # is never DMA-starved into the attention phase.
            qkv_rhs = lambda k: xT_sb[:, k * TPC:(k + 1) * TPC]
            for _rep in range(2):
                nc.scalar.dma_start(cos2_sb[:, _rep * TPC:(_rep + 1) * TPC], cos_d[:])
                nc.scalar.dma_start(sin2_sb[:, _rep * TPC:(_rep + 1) * TPC], sin_d[:])
            gemm(wqkv_src, NH + 2 * NKV, KQ, qkv_rhs, "acct", TPC, qkv_consume,
                 SW, KG, 4, sweep_starts=[16], engs=[nc.sync])

            # ---------------- KV AllGather (split shallow/deep) ----------------
            # Two collectives: each core's shallow block (kb 0..7 = even AG
            # slots) first, deep block second — the qb0 attention passes need
            # only the shallow half, so they can start while the deep half is
            # still on the wire.
            KSZ2 = NKV * BLK * BLK  # 65536 elems per half
            kv_sh = dram.tile([2, KSZ2], BF16)
            kv_dp = dram.tile([2, KSZ2], BF16)
            ag_sh = dram.tile([NC_, 2, KSZ2], BF16, addr_space="Shared")
            ag_dp = dram.tile([NC_, 2, KSZ2], BF16, addr_space="Shared")
            kv_loc = [kv_sh, kv_dp]
            for half in range(2):
                nc.gpsimd.dma_start(
                    kv_loc[half][0, :].rearrange("(h d t) -> d h t", h=NKV, d=BLK),
                    k_sb[:].rearrange("d (h t) -> d h t", h=NKV)[:, :, half * BLK:(half + 1) * BLK])
                nc.gpsimd.dma_start(
                    kv_loc[half][1, :].rearrange("(h t d) -> t h d", h=NKV, t=BLK),
                    v_sb[:].rearrange("t (b h d) -> b t h d", b=2, h=NKV)[half])
            nc.gpsimd.collective_compute(
                "AllGather", mybir.AluOpType.bypass,
                replica_groups=[list(range(NC_))],
                ins=[kv_sh[:]], outs=[ag_sh[:]])
            nc.gpsimd.collective_compute(
                "AllGather", mybir.AluOpType.bypass,
                replica_groups=[list(range(NC_))],
                ins=[kv_dp[:]], outs=[ag_dp[:]])
            # remaining qkv sweeps (q heads) overlap the collective
            gemm(wqkv_src, NH + 2 * NKV, KQ, qkv_rhs, "acct", TPC, qkv_consume,
                 SW, KG, 4, sweep_starts=[0, 8], engs=[nc.scalar])
            # deferred resident loads (needed by MoE / wo, not by qkv)
            for e in range(EPC):
                nc.scalar.dma_start(
                    xg_sb[:, e * KQ * cap:(e + 1) * KQ * cap].rearrange("p (k t) -> p k t", k=KQ),
                    xg_d[e].rearrange("(k p) t -> p k t", p=BLK))
                nc.scalar.dma_start(ew_sb[:, e * cap:(e + 1) * cap], ew_d[e])

            # ------------------- MoE sweeps (as thunks) -------------------
            def make_gu_thunks(e):
                thunks = []

                def gu_consume(ms, paps, pts, e=e):
                    for j in range(0, len(ms), 2):
                        p = (ms[0] + j) // 2
                        sg = small.tile([BLK, cap], BF16, tag="sg")
                        # psums are 32x (fp8 weight scale); silu(g) needs
                        # scale 1/32; the 32x on u is folded into ew.
                        nc.scalar.activation(sg[:], paps[j], AF.Silu, scale=1.0 / WS8)
                        nc.vector.tensor_mul(
                            hm_sb[:, (e * MI + p) * cap:(e * MI + p + 1) * cap],
                            sg[:], paps[j + 1])

                ws_src = lambda kg0, kgn, c0, c1, e=e: wsT_d[e, kg0 * BLK:(kg0 + kgn) * BLK, c0:c1] \
                    .rearrange("(k p) c -> p k c", p=BLK)
                xgv = xg_sb[:, e * KQ * cap:(e + 1) * KQ * cap] \
                    .rearrange("p (k t) -> p k t", k=KQ)
                rhs_e = lambda kp, xgv=xgv: xgv[:, 2 * kp:2 * kp + 2, :]
                for s0 in range(0, 2 * MI, 2):
                    thunks.append(lambda s0=s0, f=gu_consume, w=ws_src, r=rhs_e: gemm(
                        w, 2 * MI, KQ, r, "macct", cap, f, 2, 8, 2, sweep_starts=[s0],
                        engs=[nc.sync], dr=True))
                return thunks

            def make_w2s_thunks(e, tag, bufs, sweep):
                thunks = []

                def w2s_consume(ms, paps, pts, e=e):
                    for m, ps in zip(ms, paps):
                        mo = outp.tile([BLK, cap], BF16, tag="mo")
                        nc.vector.tensor_mul(mo[:], ps, ew_sb[:, e * cap:(e + 1) * cap])
                        eng_rr[m % 2].dma_start(moe_out_d[e, m * BLK:(m + 1) * BLK, :], mo[:])

                w2s_src = lambda kg0, kgn, c0, c1, e=e: w2sT_d[e, kg0 * BLK:(kg0 + kgn) * BLK, c0:c1] \
                    .rearrange("(k p) c -> p k c", p=BLK)
                for s0 in range(0, KQ, sweep):
                    thunks.append(lambda s0=s0, e=e, f=w2s_consume, w=w2s_src: gemm(
                        w, KQ, MI,
                        lambda k, e=e: hm_sb[:, (e * MI + k) * cap:(e * MI + k + 1) * cap],
                        tag, cap, f, sweep, 8, bufs, sweep_starts=[s0]))
                return thunks

            # both experts' gu inline before attention (silu done before any
            # exp act -> no act-table churn inside the attention phase)
            for th in make_gu_thunks(0):
                th()
            for th in make_gu_thunks(1):
                th()

            # unpack AG results on the gpsimd SWDGE ring (doesn't block the
            # HWDGE weight streams). slot = 2c+sub; attention maps kb->slot.
            ags = [ag_sh, ag_dp]
            for sub in range(2):
                for hh in range(NKV):
                    nc.gpsimd.dma_start(
                        kag_sb[:].rearrange("d (c s h t) -> s h d c t", c=NC_, s=2, h=NKV)[sub, hh],
                        ags[sub][:, 0, :].rearrange("c (h d t) -> h d c t", h=NKV, d=BLK)[hh])
                    nc.gpsimd.dma_start(
                        vag_sb[:].rearrange("t (c s h dd) -> s h t c dd", c=NC_, s=2, h=NKV)[sub, hh],
                        ags[sub][:, 1, :].rearrange("c (h t dd) -> h t c dd", h=NKV, t=BLK)[hh])

            # keep the PE clock warm through the deep-AG / unpack wait so the
            # first attention scores don't run at the throttled 1.2GHz
            for _w in range(32):
                wp2 = psum.tile([BLK, BLK], F32, tag="macct", bufs=2, name="wp2")
                nc.tensor.matmul(wp2[:], wrm[:], wrm[:], start=True, stop=True)

            # interleave thunks: w2s for both experts across attention
            # (consumes are vector-only -> scalar engine stays exp-only)
            fill = make_w2s_thunks(0, "macct", 2, 2) + make_w2s_thunks(1, "macct", 2, 2)

            # ---------------- attention: 4 kv-groups x 2 qb passes ----------------
            # pass (g, qb): 4 heads 4g..4g+3, queries = qb block (128 tokens).
            # dense kbs: qb0 -> kb 0..7, qb1 -> kb 0..15; diag = own block.
            # one pinned psum bank accumulates all 8 passes' denominators,
            # pass p landing on partition row p (one-hot stationary), so a
            # single [8,512] reciprocal serves the whole attention phase.
            dps = psum.tile([BLK, GW], F32, tag="dps", bufs=1)

            def attn_pass(g, qb, p):
                kbs = list(range(NC_ if qb == 0 else NBLK))
                nkb = len(kbs) + 1  # + diag
                qv = q_sb[:, qb * NH * BLK + 4 * g * BLK: qb * NH * BLK + 4 * (g + 1) * BLK]
                aps = psum.tile([BLK, GW], F32, tag="aps", bufs=1)
                for idx in range(nkb):
                    diag = idx == nkb - 1
                    sc = psum.tile([BLK, GW], F32, tag="acct", bufs=4, name="sc")
                    if diag:
                        kap = k_sb[:, g * TPC + qb * BLK: g * TPC + (qb + 1) * BLK]
                        vap = v_sb[:, (qb * NKV + g) * BLK:(qb * NKV + g + 1) * BLK]
                    else:
                        sl = _slot(kbs[idx])
                        kap = kag_sb[:, (sl * NKV + g) * BLK:(sl * NKV + g + 1) * BLK]
                        vap = vag_sb[:, (sl * NKV + g) * BLK:(sl * NKV + g + 1) * BLK]
                    nc.tensor.matmul(sc[:], kap, qv, start=True, stop=True)
                    pdn = small.tile([BLK, GW], BF16, tag="pdn", bufs=6)
                    if diag:
                        nc.scalar.activation(pdn[:], sc[:], AF.Exp, scale=SCALE)
                        nc.vector.tensor_mul(pdn[:], pdn[:], tri4_sb[:])
                    else:
                        nc.scalar.activation(pdn[:], sc[:], AF.Exp, scale=SCALE,
                                             bias=bias_sb[:, qb * NBLK + kbs[idx]:
                                                          qb * NBLK + kbs[idx] + 1])
                    nc.tensor.matmul(aps[:], vap, pdn[:], start=(idx == 0),
                                     stop=(idx == nkb - 1))
                    nc.tensor.matmul(dps[0:8, :], oh_sb[:, 8 * p:8 * p + 8], pdn[:],
                                     start=(p == 0 and idx == 0),
                                     stop=(p == 7 and idx == nkb - 1))
                # drain unnormalized; normalize batched after all passes
                nc.vector.tensor_copy(
                    attnU_sb[:, qb * NH * BLK + 4 * g * BLK: qb * NH * BLK + 4 * (g + 1) * BLK],
                    aps[:])

            # shallow (qb0) passes first — they only need ag_sh — then deep;
            # 12 w2s thunks inside attention, 4 left for the wo ramp.
            fi = 0
            for g in range(4):
                fill[fi](); fi += 1
                attn_pass(g, 0, g)
            for g in range(4):
                fill[fi](); fi += 1
                fill[fi](); fi += 1
                attn_pass(g, 1, 4 + g)

            # ---------------- batched softmax normalization ----------------
            rec_all = small.tile([8, GW], BF16, tag="rec")
            with nc.allow_low_precision(reason="softmax denom recip, ~2^-9 rel"):
                nc.vector.reciprocal(rec_all[:], dps[0:8, :])
            for p in range(8):
                g, qb = (p, 0) if p < 4 else (p - 4, 1)
                bcp = psum.tile([BLK, GW], F32, tag="acct", bufs=4, name="bcp")
                nc.tensor.matmul(bcp[:], sel_sb[:, p * BLK:(p + 1) * BLK],
                                 rec_all[:], start=True, stop=True)
                bcs = small.tile([BLK, GW], F32, tag="bcs")
                nc.vector.tensor_copy(bcs[:], bcp[:])
                au = attnU_sb[:, qb * NH * BLK + 4 * g * BLK:
                              qb * NH * BLK + 4 * (g + 1) * BLK]
                nc.vector.tensor_mul(au, au, bcs[:])

            # ---------------- wo + residual ----------------
            def wo_consume(ms, paps, pts):
                for jt, pt in enumerate(pts):
                    m0 = ms[2 * jt]
                    w = pt.shape[1]
                    nc.vector.tensor_add(resid_sb[:, m0 * TPC: m0 * TPC + w],
                                         pt[:], xT_sb[:, m0 * TPC: m0 * TPC + w])

            wo_src = lambda kg0, kgn, c0, c1: wo_d[kg0 * BLK:(kg0 + kgn) * BLK, c0:c1] \
                .rearrange("(k p) c -> p k c", p=BLK)
            wo_rhs = lambda k: attnU_sb[:].rearrange(
                "p (qb h t) -> p h qb t", qb=2, h=NH)[:, k]
            for i, s0 in enumerate(range(0, KQ, SW)):
                if fi < len(fill):
                    fill[fi](); fi += 1
                gemm(wo_src, KQ, KQ, wo_rhs, "acct", TPC, wo_consume, SW, KG, 4,
                     sweep_starts=[s0])
                if fi < len(fill):
                    fill[fi](); fi += 1
            while fi < len(fill):
                fill[fi](); fi += 1

            # ---------------- residual MLP norm scale ----------------
            ssq = psum.tile([BLK, TPC], F32, tag="macct", bufs=2, name="ssq")
            for k in range(KQ):
                sq = small.tile([BLK, TPC], F32, tag="sq")
                nc.vector.tensor_mul(sq[:], resid_sb[:, k * TPC:(k + 1) * TPC],
                                     resid_sb[:, k * TPC:(k + 1) * TPC])
                nc.tensor.matmul(ssq[0:1, :], ones_cf[:], sq[:],
                                 start=(k == 0), stop=(k == KQ - 1))
            vtmp2 = small.tile([1, TPC], F32, tag="vt")
            nc.vector.tensor_scalar(vtmp2[:], ssq[0:1, :], 1.0 / H, EPS,
                                    mybir.AluOpType.mult, mybir.AluOpType.add)
            st = small.tile([1, TPC], F32, tag="vt2")
            nc.scalar.activation(st[:], vtmp2[:], AF.Sqrt)
            s2r = small.tile([1, TPC], BF16, tag="vt3")
            with nc.allow_low_precision(reason="rms-norm scale recip, ~2^-9 rel"):
                nc.vector.reciprocal(s2r[:], st[:])
            s2p = psum.tile([BLK, TPC], F32, tag="macct", bufs=2, name="s2p")
            nc.tensor.matmul(s2p[:], ones_row[:], s2r[:], start=True, stop=True)
            s2s = small.tile([BLK, TPC], F32, tag="s2s")
            nc.scalar.activation(s2s[:], s2p[:], AF.Copy)
            for k in range(KQ):
                nc.vector.tensor_mul(h2_sb[:, k * TPC:(k + 1) * TPC],
                                     resid_sb[:, k * TPC:(k + 1) * TPC], s2s[:])

            # ---------------- w13 (interleaved g/u) + silu_and_mul ----------------
            def w13_consume(ms, paps, pts):
                for jt, pt in enumerate(pts):
                    p = ms[2 * jt] // 2
                    sg = small.tile([BLK, TPC], BF16, tag="sg13")
                    nc.scalar.activation(sg[:], pt[:, 0:TPC], AF.Silu)
                    nc.vector.tensor_mul(gu_sb[:, p * TPC:(p + 1) * TPC],
                                         sg[:], pt[:, TPC:2 * TPC])

            w13_src = lambda kg0, kgn, c0, c1: w13_d[kg0 * BLK:(kg0 + kgn) * BLK, c0:c1] \
                .rearrange("(k p) c -> p k c", p=BLK)
            gemm(w13_src, 2 * KQ, KQ, lambda k: h2_sb[:, k * TPC:(k + 1) * TPC],
                 "acct", TPC, w13_consume, SW, KG, 4)

            # ---------------- w2 + final out ----------------
            def w2_consume(ms, paps, pts):
                for jt, pt in enumerate(pts):
                    m0 = ms[2 * jt]
                    w = pt.shape[1]
                    fo = outp.tile([BLK, 2 * TPC], F32, tag="fo")
                    nc.vector.tensor_add(fo[:, :w], pt[:], resid_sb[:, m0 * TPC:m0 * TPC + w])
                    nc.sync.dma_start(
                        res_out_d.ap().rearrange("(m p) t -> p m t", p=BLK)[:, m0:m0 + w // TPC],
                        fo[:, :w].rearrange("p (m t) -> p m t", t=TPC))

            w2_src = lambda kg0, kgn, c0, c1: w2_d[kg0 * BLK:(kg0 + kgn) * BLK, c0:c1] \
                .rearrange("(k p) c -> p k c", p=BLK)
            gemm(w2_src, KQ, KQ, lambda k: gu_sb[:, k * TPC:(k + 1) * TPC],
                 "acct", TPC, w2_consume, SW, KG, 4)

    nc.compile()
    return nc


def _interleave_cols(w, half):
    # [rows, 2*half] -> column chunks reordered so chunk 2p=g_p, 2p+1=u_p
    rows = w.shape[0]
    g = w[:, :half].reshape(rows, half // BLK, BLK)
    u = w[:, half:].reshape(rows, half // BLK, BLK)
    out = np.empty((rows, 2 * (half // BLK), BLK), w.dtype)
    out[:, 0::2] = g
    out[:, 1::2] = u
    return out.reshape(rows, 2 * half // BLK * BLK)


def kernel(**inputs):
    global LAST_RESULT
    hidden = f32(inputs["hidden_states"])
    positions = np.asarray(inputs["positions"]).astype(np.float32)
    ln_in_w = f32(inputs["ln_in_w"])
    ln_post_w = f32(inputs["ln_post_w"])
    ln_res_w = f32(inputs["ln_res_w"])
    wqkv = f32(inputs["wqkv"])
    wo = f32(inputs["wo"])
    res_w13 = f32(inputs["res_w13"])
    res_w2 = f32(inputs["res_w2"])
    gate_w = f32(inputs["gate_w"])
    ws = f32(inputs["ws"])
    w2s = f32(inputs["w2s"])

    # ---- host prep (sharding) ----
    s = 1.0 / np.sqrt(np.mean(hidden * hidden, axis=1) + EPS)  # [T]
    x_norm = hidden * s[:, None]

    logits = (x_norm * ln_post_w) @ gate_w
    pr = np.exp(logits - logits.max(-1, keepdims=True))
    pr /= pr.sum(-1, keepdims=True)
    topi = np.argsort(-pr, axis=-1, kind="stable")[:, :TOPK]
    topw = np.take_along_axis(pr, topi, axis=-1)
    topw /= topw.sum(-1, keepdims=True)
    tok_lists = [np.where((topi == e).any(-1))[0] for e in range(E)]
    wts = [np.sum(np.where(topi[tl] == e, topw[tl], 0.0), -1).astype(np.float32)
           for e, tl in zip(range(E), tok_lists)]
    cap = max(128, -(-max(len(t) for t in tok_lists) // 32) * 32)
    assert cap <= 512, cap

    ck = cap
    if ck not in _CACHE:
        _CACHE[ck] = _build(cap)
    nc = _CACHE[ck]

    inv_freq = 1.0 / (THETA ** (np.arange(0, HD, 2, dtype=np.float32) / HD))
    ang = positions[:, None] * inv_freq
    cos_t, sin_t = np.cos(ang), np.sin(ang)

    tri01 = (np.arange(BLK)[None, :] >= np.arange(BLK)[:, None]).astype(np.float32)
    tri4 = np.tile(tri01, (1, 4))
    ident = np.eye(BLK, dtype=np.float32)
    ohtab = np.zeros((BLK, 64), np.float32)
    for p in range(8):
        ohtab[:, p * 8 + p] = 1.0
    seltab = np.zeros((8, 8 * BLK), np.float32)
    for p in range(8):
        seltab[p, p * BLK:(p + 1) * BLK] = 1.0

    wqkv_f = wqkv * ln_in_w[:, None]
    w13_f = _interleave_cols(res_w13 * ln_res_w[:, None], H)
    x_norm_post = x_norm * ln_post_w
    wsT = ws.transpose(0, 2, 1)  # [E, H, 2I]
    wsT_il = np.stack([_interleave_cols(wsT[e], I) for e in range(E)])
    w2sT = w2s.transpose(0, 2, 1)

    shared = {
        "tri4": bf(tri4), "ident": bf(ident),
        "ohtab": bf(ohtab), "seltab": bf(seltab),
        "wqkv": bf(wqkv_f), "wo": bf(wo), "w13": bf(w13_f), "w2": bf(res_w2),
    }

    in_maps = []
    own = [[i, NBLK - 1 - i] for i in range(NC_)]
    for i in range(NC_):
        toks = np.concatenate([np.arange(b * BLK, (b + 1) * BLK) for b in own[i]])
        xT = hidden[toks].T
        cs = (cos_t[toks] * s[toks, None]).T
        sn = (sin_t[toks] * s[toks, None]).T
        scol = np.stack([s[toks[:BLK]], s[toks[BLK:]]], axis=1)
        bias = np.zeros((2, NBLK, BLK), np.float32)
        b0, b1 = own[i]
        bias[0, b0:, :] = NEG
        bias[1, b1:, :] = NEG
        exps = [2 * i, 2 * i + 1]
        xg = np.zeros((EPC, H, cap), np.float32)
        ew = np.zeros((EPC, BLK, cap), np.float32)
        for j, e in enumerate(exps):
            n = len(tok_lists[e])
            xg[j, :, :n] = x_norm_post[tok_lists[e]].T
            # ws runs in fp8 with weights x32; u-path 32x folded here
            ew[j, :, :n] = wts[e][None, :] / WS8
        in_maps.append({
            "xT_bf": bf(xT),
            "cos_s": f32(cs), "sin_s": f32(sn), "s_col": f32(scol),
            "bias": bias,
            "wsT": f8(wsT_il[exps] * WS8),
            "w2sT": bf(w2sT[exps]),
            "xgT": f8(xg), "ew": ew,
            **shared,
        })

    res = run_bass_kernel_spmd(nc, in_maps, core_ids=list(range(NC_)), trace=TRACE)
    LAST_RESULT = res

    out = np.zeros((T, H), np.float32)
    for i in range(NC_):
        toks = np.concatenate([np.arange(b * BLK, (b + 1) * BLK) for b in own[i]])
        out[toks] = res.results[i]["res_out"].T
    for i in range(NC_):
        for j, e in enumerate((2 * i, 2 * i + 1)):
            tl = tok_lists[e]
            out[tl] += res.results[i]["moe_out"][j].astype(np.float32).T[:len(tl)]
    return out
